# revision 34
# baseline (speedup 1.0000x reference)
"""Trainium2 Bass kernel for MultiHeadLinearSelfAttention (linear attention +
LePE depthwise conv + projections), SPMD data-parallel over batch on 8 cores.

v2 design (measured rel err 0.0058 vs 2e-2 gate; cost-model exec 168us/core
vs 215us baseline):

Precision split (LePE dominates output RMS 8.5:1 over attention):
  - LePE path bf16 end-to-end: v channel-major production, 3x3 taps,
    rat, out_proj; y output bf16.
  - Attention path fp8e4m3 with DoubleRow matmuls (2x PE rate): q/k
    production from an fp8 copy of x, kv accumulation (v pre-scaled 1/4),
    den (ksum/512, 4x-replicated stationary for the 32-partition ISA
    minimum), q-hat normalization, and the at = blockdiag(kv) @ qh matmul
    (qh scaled 2^16/den via em=128 and ksum/512; compensated by 2^-14 at
    the rat evacuation; LePE diag weights pre-scaled 2^14 to share the
    at-psum accumulation).

Engine balance (per core ~112us PE / ~103 DVE / ~99 ACT):
  - PE: all DR matmuls + bf16 vch/bc/out_proj + 8 of 9 LePE diag taps
    accumulated directly into at-psum (taps are emitted inside the at
    accumulation group, before the qh-dependent DR matmul).
  - ACT: Exp (k and q, q with fused per-partition bias), vsb 1/4-scale
    fp8 evacuation, vpad fill (+v bias), output bias evacuation.
  - DVE: custom fused op ELU1_BIAS_ANT: out = max(in0+s0,0)+min(in1,1)
    (one op for the elu tail, bias folded for q), qh mul, rat
    (at*2^-14 + lep STT), reciprocal, tap 0 as 4x tensor_scalar_mul.
  - Pool/GPSIMD: memsets + x DMAs only (hardware cannot touch PSUM from
    GPSIMD, and TensorScalarPtr STT/AP-scalar forms are PE/DVE-only).

Scheduling: per image, phase A (16 pair-iters: k/v pixel-major DR, elu,
kv accumulation, interleaved vch->vpad) then B (bd/KS extraction) emitted
before the next image's phase A (kv psum bufs=1 would otherwise stall PE);
phase C (8 chunks x 2 images interleaved). x is loaded as split
bf16/fp8 half-tiles so compute starts before the full image arrives.
"""

import os
import sys

for _p in ("/opt/trn_rl_repo",):
    if _p not in sys.path:
        sys.path.insert(0, _p)

import numpy as np
import ml_dtypes

import concourse.bass as bass
import concourse.bacc as bacc
import concourse.mybir as mybir
import concourse.tile as tile
from concourse.bass_utils import run_bass_kernel_spmd
from concourse import dve_ops as _dvo
from concourse.dve_spec import Spec as _Spec, Src0, Src1, C0, Zero, One, maxx, minn


def _register_elu_op():
    """Fused elu(x+b)+1 tail: out = max(in0+s0, 0) + min(in1, 1)."""
    name = "ELU1_BIAS_ANT"
    for _op in _dvo.OPS:
        if _op.name == name:
            return _op
    spec = _Spec(
        body=maxx(Src0 + C0, Zero) + minn(Src1, One),
        reference=lambda in0, in1, s0, s1, imm2:
            np.maximum(np.float32(in0) + s0, 0) + np.minimum(np.float32(in1), 1),
    )
    op = _dvo.DveOp(name, spec, subdim=False, uops_sha={})
    _dvo._SUB_OPCODE_FOR_NAME[name] = max(_dvo._SUB_OPCODE_FOR_NAME.values()) + 1
    import re
    for ver in ("v3", "v4"):
        try:
            op.compile(ver)
        except ValueError as e:
            m = re.search(ver + r': ([0-9a-f]+)', str(e))
            if m:
                op.uops_sha[ver] = m.group(1)
    _dvo.OPS.append(op)
    _dvo.CUSTOM_DVE_SPECS[name] = spec
    return op


ELU1 = _register_elu_op()

BF16 = mybir.dt.bfloat16
F32 = mybir.dt.float32
F8 = mybir.dt.float8e4
AF = mybir.ActivationFunctionType
ALU = mybir.AluOpType
DR = mybir.MatmulPerfMode.DoubleRow

N_CORES = 8
IMG = 2            # images per core (B=16)
C = 256
N = 4096           # pixels (64x64)
TAPS = [(ty - 1, tx - 1) for ty in range(3) for tx in range(3)]

# tap assignment: which tap indices run on PE (diag matmuls into at-psum,
# weights pre-scaled by 2^14) vs DVE (STT on [128,4096] lep tiles).
PE_TAPS = tuple(int(t) for t in os.environ.get("BK_PE_TAPS", "1,2,3,4,5,6,7,8").split(",") if t != "")
POOL_TAPS = tuple(int(t) for t in os.environ.get("BK_POOL_TAPS", "").split(",") if t != "")
DVE_TAPS = tuple(t for t in range(9) if t not in PE_TAPS and t not in POOL_TAPS)

_CACHE = {}


def build_program():
    nc = bacc.Bacc(
        "TRN2", target_bir_lowering=False, debug=False,
        enable_asserts=False, num_devices=N_CORES,
    )
    xb_d = nc.dram_tensor("xb", [IMG, 128, 8192], BF16, kind="ExternalInput").ap()
    xf_d = nc.dram_tensor("xf", [IMG, 128, 8192], F8, kind="ExternalInput").ap()
    w8_d = nc.dram_tensor("w8", [128, 1536], F8, kind="ExternalInput").ap()
    row8_d = nc.dram_tensor("row8", [1, 2560], F8, kind="ExternalInput").ap()
    wb_d = nc.dram_tensor("wb", [128, 3346], BF16, kind="ExternalInput").ap()
    em_d = nc.dram_tensor("em", [8, 256], BF16, kind="ExternalInput").ap()
    bcol_d = nc.dram_tensor("bcol", [128, 24], F32, kind="ExternalInput").ap()
    y_d = nc.dram_tensor("y", [IMG, C, N], BF16, kind="ExternalOutput").ap()

    with tile.TileContext(nc) as tc:
        with (
            tc.tile_pool(name="const", bufs=1) as const,
            tc.tile_pool(name="sb", bufs=1) as sb,
            tc.tile_pool(name="ps", bufs=1, space=bass.MemorySpace.PSUM) as ps,
        ):
            # ---------------- constants ----------------
            w8 = const.tile([128, 1536], F8, name="w8", tag="w8")
            nc.scalar.dma_start(w8[:], w8_d[:])
            wb = const.tile([128, 3346], BF16, name="wb", tag="wb")
            nc.scalar.dma_start(wb[:, 0:512], wb_d[:, 0:512])
            w83 = w8[:].rearrange("p (s f) -> p s f", s=2)
            wq8 = w83[:, :, 0:256]
            wk8 = w83[:, :, 256:512]
            wv8 = w83[:, :, 512:768]

            row8 = const.tile([1, 2560], F8, name="row8", tag="row8")
            nc.sync.dma_start(row8[:], row8_d[:])
            row83 = row8[:].rearrange("p (s f) -> p s f", s=2)
            ones8 = row83[:, :, 0:512]       # s0=1, s1=0
            bq8 = row83[:, :, 512:768]       # s0=bq, s1=0
            bk8 = row83[:, :, 768:1280]      # s0=[bk|bk], s1=0

            wvb = [wb[:, 0:256], wb[:, 256:512]]
            wo = [wb[:, 512:768], wb[:, 768:1024]]

            def diag(t, g):
                o = 1024 + (t * 2 + g) * 128
                return wb[:, o:o + 128]

            emt = const.tile([8, 256], BF16, name="emt", tag="emt")
            nc.sync.dma_start(emt[:], em_d[:])
            bct = const.tile([128, 24], F32, name="bct", tag="bct")
            nc.sync.dma_start(bct[:], bcol_d[:])
            bvc = [bct[:, 0:1], bct[:, 1:2]]
            btc = [bct[:, 2:3], bct[:, 3:4]]
            bqc = [bct[:, 22:23], bct[:, 23:24]]

            def wcol(t, g):
                o = 4 + t * 2 + g
                return bct[:, o:o + 1]

            ones4 = const.tile([128, 4], BF16, name="ones4", tag="ones4")
            nc.gpsimd.memset(ones4[:], 1.0)

            st = {}  # per-image state

            def load_x(u, img0):
                # split by pixel half, interleaved xf/xb so compute starts early
                xfd3 = xf_d[img0].rearrange("p (s f) -> p s f", s=2)
                xbd3 = xb_d[img0].rearrange("p (s f) -> p s f", s=2)
                xfh, xbh = [], []
                for hlf in range(2):
                    px = slice(2048 * hlf, 2048 * hlf + 2048)
                    tf = sb.tile([128, 4096], F8, name=f"xf{u}_{hlf}",
                                 tag=f"xf{hlf}", bufs=2)
                    nc.gpsimd.dma_start(
                        tf[:].rearrange("p (s f) -> p s f", s=2), xfd3[:, :, px])
                    xfh.append(tf[:].rearrange("p (s f) -> p s f", s=2))
                    tb = sb.tile([128, 4096], BF16, name=f"xb{u}_{hlf}",
                                 tag=f"xb{hlf}", bufs=2)
                    nc.gpsimd.dma_start(
                        tb[:].rearrange("p (s f) -> p s f", s=2), xbd3[:, :, px])
                    xbh.append(tb[:].rearrange("p (s f) -> p s f", s=2))
                if u == 0:
                    nc.scalar.dma_start(wb[:, 512:3346], wb_d[:, 512:3346])
                st[u] = {"xfh": xfh, "xbh": xbh}

                def xf3s(px0, n):
                    h = px0 // 2048
                    assert (px0 + n - 1) // 2048 == h
                    return xfh[h][:, :, px0 - 2048 * h:px0 - 2048 * h + n]

                def xb3s(gin, px0, n):
                    h = px0 // 2048
                    return xbh[h][:, gin, px0 - 2048 * h:px0 - 2048 * h + n]

                st[u]["xf3s"] = xf3s
                st[u]["xb3s"] = xb3s

            def phase_a(u):
                s_ = st[u]
                xf3s, xb3s = s_["xf3s"], s_["xb3s"]
                kvp = [ps.tile([128, 257], F32, name=f"kv{u}_{g}",
                               tag=f"kv{g}", bufs=1) for g in range(2)]
                s_["kvp"] = kvp
                vpad = []
                for g in range(2):
                    t = sb.tile([128, 66 * 66], BF16, name=f"vp{u}_{g}",
                                tag=f"vp{g}", bufs=2)
                    t3 = t[:].rearrange("p (r x) -> p r x", x=66)
                    # zero only the pad ring (interior is overwritten)
                    nc.gpsimd.memset(t3[:, 0:1, :], 0.0)
                    nc.gpsimd.memset(t3[:, 65:66, :], 0.0)
                    nc.gpsimd.memset(t3[:, 1:65, 0:1], 0.0)
                    nc.gpsimd.memset(t3[:, 1:65, 65:66], 0.0)
                    vpad.append(t)
                s_["vpad"] = vpad

                for j in range(16):
                    # ---- k pixel-major (fp8 DR), 2 chunks of 128 px ----
                    kp = ps.tile([128, 512], F32, name=f"kp{u}_{j}", tag="kp", bufs=3)
                    for h in range(2):
                        px0 = 128 * (2 * j + h)
                        nc.tensor.matmul(kp[:, 256 * h:256 * h + 256],
                                         xf3s(px0, 128), wk8,
                                         start=(h == 0), stop=False, perf_mode=DR)
                    nc.tensor.matmul(kp[:], ones8[:, :, 0:128], bk8,
                                     start=False, stop=True, perf_mode=DR)
                    ek = sb.tile([128, 512], BF16, name=f"ek{u}_{j}", tag="ek", bufs=6)
                    nc.scalar.activation(ek[:], kp[:], AF.Exp)
                    kh = sb.tile([128, 512], F8, name=f"kh{u}_{j}", tag="kh", bufs=6)
                    nc.vector._custom_dve(ELU1, out=kh[:], in0=kp[:], in1=ek[:])
                    kh3 = kh[:].rearrange("p (s f) -> p s f", s=2)

                    # ---- v pixel-major (fp8 DR) ----
                    vp = ps.tile([128, 512], F32, name=f"vq{u}_{j}", tag="vq", bufs=2)
                    for h in range(2):
                        px0 = 128 * (2 * j + h)
                        nc.tensor.matmul(vp[:, 256 * h:256 * h + 256],
                                         xf3s(px0, 128), wv8,
                                         start=(h == 0), stop=(h == 1), perf_mode=DR)
                    vsb = sb.tile([128, 514], F8, name=f"vs{u}_{j}", tag="vs", bufs=6)
                    vsb3 = vsb[:].rearrange("p (s f) -> p s f", s=2)
                    nc.gpsimd.memset(vsb3[:, :, 0:1], 1.0)
                    nc.scalar.activation(vsb3[:, :, 1:257], vp[:], AF.Identity,
                                         scale=0.25)

                    # ---- v channel-major (bf16) for LePE; one (g, c8) per j ----
                    g, c8 = j // 8, j % 8
                    vc = ps.tile([128, 512], F32, name=f"vc{u}_{j}", tag="vc", bufs=1)
                    for gin in range(2):
                        nc.tensor.matmul(vc[:], wvb[gin][:, 128 * g:128 * g + 128],
                                         xb3s(gin, 512 * c8, 512),
                                         start=(gin == 0), stop=(gin == 1))
                    vp3 = vpad[g][:].rearrange("p (r x) -> p r x", x=66)
                    dst = vp3[:, 8 * c8 + 1:8 * c8 + 9, 1:65]
                    nc.scalar.activation(dst, vc[:], AF.Identity, bias=bvc[g])

                    # ---- kv accumulation (fp8 DR over 256 px) ----
                    for g in range(2):
                        nc.tensor.matmul(kvp[g][:], kh3[:, :, 128 * g:128 * g + 128],
                                         vsb3, start=(j == 0), stop=(j == 15),
                                         perf_mode=DR)

                # ---- DVE lepe taps on [128, 4096] tiles ----
                if not DVE_TAPS and not POOL_TAPS:
                    s_["lep"] = None
                    return
                lep = []
                for g in range(2):
                    t = sb.tile([128, 4096], BF16, name=f"lp{u}_{g}",
                                tag=f"lp{g}", bufs=2)
                    t3 = t[:].rearrange("p (r x) -> p r x", x=64)
                    vp3 = vpad[g][:].rearrange("p (r x) -> p r x", x=66)
                    first = True
                    for ti in DVE_TAPS:
                        dy, dx = TAPS[ti]
                        win = vp3[:, dy + 1:dy + 65, dx + 1:dx + 65]
                        if first:
                            nc.vector.tensor_scalar_mul(t3[:], win, wcol(ti, g))
                            first = False
                        else:
                            nc.vector.scalar_tensor_tensor(t3[:], win, wcol(ti, g),
                                                           t3[:], ALU.mult, ALU.add)
                    for ti in POOL_TAPS:
                        dy, dx = TAPS[ti]
                        win = vp3[:, dy + 1:dy + 65, dx + 1:dx + 65]
                        nc.gpsimd.scalar_tensor_tensor(t3[:], win, wcol(ti, g),
                                                       t3[:], ALU.mult, ALU.add)
                    lep.append(t)
                s_["lep"] = lep

            def phase_b(u):
                s_ = st[u]
                kvp = s_["kvp"]
                bd = []
                for g in range(2):
                    t = sb.tile([128, 256], F8, name=f"bd{u}_{g}", tag=f"bd{g}", bufs=2)
                    nc.gpsimd.memset(t[:], 0.0)
                    t3 = t[:].rearrange("p (s f) -> p s f", s=2)
                    for h in range(4):
                        sl = slice(32 * h, 32 * h + 32)
                        c0 = 1 + 128 * g + 32 * h
                        nc.vector.tensor_copy(t3[sl, g, sl], kvp[g][sl, c0:c0 + 32])
                    bd.append(t)
                KS = sb.tile([128, 64], F8, name=f"KS{u}", tag="KS", bufs=2)
                nc.gpsimd.memset(KS[:], 0.0)
                KS4 = KS[:].rearrange("p (s r h) -> p s r h", s=2, h=8)
                for g in range(2):
                    for h4 in range(4):
                        sl = slice(32 * h4, 32 * h4 + 32)
                        h = 4 * g + h4
                        nc.vector.tensor_scalar(KS4[sl, g, :, h], ones4[sl, :],
                                                kvp[g][sl, 0:1], 1.0 / 512.0,
                                                ALU.mult, ALU.mult)
                s_["bd"], s_["KS"] = bd, KS

            def c_iter(u, img0, nt):
                s_ = st[u]
                xf3s = s_["xf3s"]
                bd, KS, lep, vpad = s_["bd"], s_["KS"], s_["lep"], s_["vpad"]
                KS3 = KS[:].rearrange("p (s f) -> p s f", s=2)  # [128, 2, 32]
                px = slice(512 * nt, 512 * nt + 512)

                S2 = sb.tile([128, 1024], F8, name=f"S{u}_{nt}", tag="S2", bufs=4)
                S23 = S2[:].rearrange("p (s f) -> p s f", s=2)
                for og in range(2):
                    qp = ps.tile([128, 512], F32, name=f"qp{u}_{og}_{nt}",
                                 tag="kp", bufs=3)
                    nc.tensor.matmul(qp[:], wq8[:, :, 128 * og:128 * og + 128],
                                     xf3s(512 * nt, 512), start=True, stop=True,
                                     perf_mode=DR)
                    eq = sb.tile([128, 512], BF16, name=f"eq{u}_{og}_{nt}",
                                 tag="eq", bufs=5)
                    nc.scalar.activation(eq[:], qp[:], AF.Exp, bias=bqc[og])
                    nc.vector._custom_dve(ELU1, out=S23[:, og, :], in0=qp[:],
                                          in1=eq[:], s0=bqc[og])

                den = ps.tile([32, 512], F32, name=f"dn{u}_{nt}", tag="kv0", bufs=1)
                nc.tensor.matmul(den[:], KS3, S23, start=True, stop=True, perf_mode=DR)
                rc = sb.tile([8, 512], BF16, name=f"rc{u}_{nt}", tag="rc", bufs=4)
                with nc.allow_low_precision(reason="recip feeds bf16 matmul"):
                    nc.vector.reciprocal(rc[:], den[0:8, :])

                qh = sb.tile([128, 1024], F8, name=f"qh{u}_{nt}", tag="qh", bufs=4)
                qh3 = qh[:].rearrange("p (s f) -> p s f", s=2)
                for g in range(2):
                    bc = ps.tile([128, 512], F32, name=f"bc{u}_{g}_{nt}",
                                 tag="kv0", bufs=1)
                    nc.tensor.matmul(bc[:], emt[:, 128 * g:128 * g + 128], rc[:],
                                     start=True, stop=True)
                    nc.vector.tensor_mul(qh3[:, g, :], S23[:, g, :], bc[:])
                rats = []
                for g in range(2):
                    bd3 = bd[g][:].rearrange("p (s f) -> p s f", s=2)
                    at = ps.tile([128, 512], F32, name=f"at{u}_{g}_{nt}",
                                 tag="vq", bufs=2)
                    vp3 = vpad[g][:].rearrange("p (r x) -> p r x", x=66)
                    for i, ti in enumerate(PE_TAPS):
                        dy, dx = TAPS[ti]
                        win = vp3[:, 8 * nt + dy + 1:8 * nt + dy + 9,
                                  dx + 1:dx + 65]
                        nc.tensor.matmul(at[:], diag(ti, g), win, start=(i == 0),
                                         stop=False)
                    nc.tensor.matmul(at[:], bd3, qh3,
                                     start=(len(PE_TAPS) == 0), stop=True,
                                     perf_mode=DR)
                    rat = sb.tile([128, 512], BF16, name=f"rt{u}_{g}_{nt}",
                                  tag=f"rt{g}", bufs=2)
                    if lep is not None:
                        nc.vector.scalar_tensor_tensor(rat[:], at[:],
                                                       1.0 / 16384.0,
                                                       lep[g][:][:, px],
                                                       ALU.mult, ALU.add)
                    else:
                        nc.vector.tensor_scalar_mul(rat[:], at[:], 1.0 / 16384.0)
                    rats.append(rat)

                for og in range(2):
                    op_ = ps.tile([128, 512], F32, name=f"op{u}_{og}_{nt}",
                                  tag="vc", bufs=1)
                    for g in range(2):
                        nc.tensor.matmul(op_[:], wo[g][:, 128 * og:128 * og + 128],
                                         rats[g][:], start=(g == 0), stop=(g == 1))
                    yt = sb.tile([128, 512], BF16, name=f"yt{u}_{og}_{nt}",
                                 tag="yt", bufs=4)
                    nc.scalar.activation(yt[:], op_[:], AF.Identity, bias=btc[og])
                    nc.sync.dma_start(
                        y_d[img0, 128 * og:128 * og + 128, px], yt[:])

            # ---------------- schedule ----------------
            us = list(range(IMG))
            for i, u in enumerate(us):
                load_x(u, i)
            for u in us:
                phase_a(u)
                phase_b(u)
            for nt in range(8):
                for i, u in enumerate(us):
                    c_iter(u, i, nt)

    nc.compile()
    return nc


def _prep_inputs(x, qkv_w, qkv_b, lepe_w, lepe_b, out_w, out_b):
    bf = ml_dtypes.bfloat16
    f8 = ml_dtypes.float8_e4m3
    x = np.ascontiguousarray(np.asarray(x, np.float32)).reshape(16, C, N)
    qkv_w = np.asarray(qkv_w, np.float32)
    qkv_b = np.asarray(qkv_b, np.float32)
    lepe_w = np.asarray(lepe_w, np.float32)
    lepe_b = np.asarray(lepe_b, np.float32)
    out_w = np.asarray(out_w, np.float32)
    out_b = np.asarray(out_b, np.float32)

    Wq, Wk, Wv = qkv_w[0:256], qkv_w[256:512], qkv_w[512:768]
    bq, bk, bv = qkv_b[0:256], qkv_b[256:512], qkv_b[512:768]

    # x in [p, s, n] layout: channel = s*128 + p
    xr = x.reshape(16, 2, 128, N).transpose(0, 2, 1, 3).reshape(16, 128, 2 * N)

    # w8: [128, 2, 768] = wq8 | wk8 | wv8, each [p, s, m] = W[m, s*128+p]
    w8 = np.zeros((128, 2, 768), np.float32)
    for s in range(2):
        w8[:, s, 0:256] = Wq[:, s * 128:(s + 1) * 128].T
        w8[:, s, 256:512] = Wk[:, s * 128:(s + 1) * 128].T
        w8[:, s, 512:768] = Wv[:, s * 128:(s + 1) * 128].T
    w8 = w8.reshape(128, 1536).astype(f8)

    # row8: [1, 2, 1280]: ones(512) | bq(256) | bkbk(512)
    row8 = np.zeros((1, 2, 1280), np.float32)
    row8[0, 0, 0:512] = 1.0
    row8[0, 0, 512:768] = bq
    row8[0, 0, 768:1280] = np.concatenate([bk, bk])
    row8 = row8.reshape(1, 2560).astype(f8)

    # wb (bf16): wvb(512) | wo(512) | diag taps(2304) | wcols(18)
    wb = np.zeros((128, 3346), np.float32)
    for gin in range(2):
        wb[:, 256 * gin:256 * gin + 256] = Wv[:, gin * 128:(gin + 1) * 128].T
        wb[:, 512 + 256 * gin:512 + 256 * gin + 256] = out_w[:, gin * 128:(gin + 1) * 128].T
    lw = lepe_w[:, 0]  # [256, 3, 3]
    for t, (dy, dx) in enumerate(TAPS):
        for g in range(2):
            o = 1024 + (t * 2 + g) * 128
            d = np.zeros((128, 128), np.float32)
            np.fill_diagonal(d, lw[g * 128:(g + 1) * 128, dy + 1, dx + 1] * 16384.0)
            wb[:, o:o + 128] = d
    wb = wb.astype(bf)

    em = np.zeros((8, 256), np.float32)
    for m in range(256):
        em[m // 32, m] = 128.0
    em = em.astype(bf)

    # v-bias columns + total output bias (v-bias in attn path folds into it)
    btot = out_b + out_w @ lepe_b + out_w @ bv
    bcol = np.zeros((128, 24), np.float32)
    bcol[:, 0] = bv[0:128]
    bcol[:, 1] = bv[128:256]
    bcol[:, 2] = btot[0:128]
    bcol[:, 3] = btot[128:256]
    for t in range(9):
        dy, dx = TAPS[t]
        for g in range(2):
            bcol[:, 4 + t * 2 + g] = lw[g * 128:(g + 1) * 128, dy + 1, dx + 1]
    bcol[:, 22] = bq[0:128]
    bcol[:, 23] = bq[128:256]

    shared = dict(w8=w8, row8=row8, wb=wb, em=em, bcol=bcol)
    in_maps = []
    for c in range(N_CORES):
        m = dict(shared)
        xc = xr[c * IMG:(c + 1) * IMG]
        m["xb"] = xc.astype(bf)
        m["xf"] = xc.astype(f8)
        in_maps.append(m)
    return in_maps


def kernel(x, qkv_w, qkv_b, lepe_w, lepe_b, out_w, out_b):
    if "nc" not in _CACHE:
        _CACHE["nc"] = build_program()
    nc = _CACHE["nc"]
    in_maps = _prep_inputs(x, qkv_w, qkv_b, lepe_w, lepe_b, out_w, out_b)
    res = run_bass_kernel_spmd(nc, in_maps, core_ids=list(range(N_CORES)))
    out = np.concatenate([np.asarray(r["y"], np.float32) for r in res.results])
    return out.reshape(16, C, 64, 64)


if __name__ == "__main__":
    build_program()
    print("build OK")


# revision 35
# speedup vs baseline: 1.0170x; 1.0170x over previous
"""Trainium2 Bass kernel for MultiHeadLinearSelfAttention (linear attention +
LePE depthwise conv + projections), SPMD data-parallel over batch on 8 cores.

v2 design (measured rel err 0.0058 vs 2e-2 gate; cost-model exec 168us/core
vs 215us baseline):

Precision split (LePE dominates output RMS 8.5:1 over attention):
  - LePE path bf16 end-to-end: v channel-major production, 3x3 taps,
    rat, out_proj; y output bf16.
  - Attention path fp8e4m3 with DoubleRow matmuls (2x PE rate): q/k
    production from an fp8 copy of x, kv accumulation (v pre-scaled 1/4),
    den (ksum/512, 4x-replicated stationary for the 32-partition ISA
    minimum), q-hat normalization, and the at = blockdiag(kv) @ qh matmul
    (qh scaled 2^16/den via em=128 and ksum/512; compensated by 2^-14 at
    the rat evacuation; LePE diag weights pre-scaled 2^14 to share the
    at-psum accumulation).

Engine balance (per core ~112us PE / ~103 DVE / ~99 ACT):
  - PE: all DR matmuls + bf16 vch/bc/out_proj + 8 of 9 LePE diag taps
    accumulated directly into at-psum (taps are emitted inside the at
    accumulation group, before the qh-dependent DR matmul).
  - ACT: Exp (k and q, q with fused per-partition bias), vsb 1/4-scale
    fp8 evacuation, vpad fill (+v bias), output bias evacuation.
  - DVE: custom fused op ELU1_BIAS_ANT: out = max(in0+s0,0)+min(in1,1)
    (one op for the elu tail, bias folded for q), qh mul, rat
    (at*2^-14 + lep STT), reciprocal, tap 0 as 4x tensor_scalar_mul.
  - Pool/GPSIMD: memsets + x DMAs only (hardware cannot touch PSUM from
    GPSIMD, and TensorScalarPtr STT/AP-scalar forms are PE/DVE-only).

Scheduling: per image, phase A (16 pair-iters: k/v pixel-major DR, elu,
kv accumulation, interleaved vch->vpad) then B (bd/KS extraction) emitted
before the next image's phase A (kv psum bufs=1 would otherwise stall PE);
phase C (8 chunks x 2 images interleaved). x is loaded as split
bf16/fp8 half-tiles so compute starts before the full image arrives.
"""

import os
import sys

for _p in ("/opt/trn_rl_repo",):
    if _p not in sys.path:
        sys.path.insert(0, _p)

import numpy as np
import ml_dtypes

import concourse.bass as bass
import concourse.bacc as bacc
import concourse.mybir as mybir
import concourse.tile as tile
from concourse.bass_utils import run_bass_kernel_spmd
from concourse import dve_ops as _dvo
from concourse.dve_spec import Spec as _Spec, Src0, Src1, C0, Zero, One, maxx, minn


def _register_elu_op():
    """Fused elu(x+b)+1 tail: out = max(in0+s0, 0) + min(in1, 1)."""
    name = "ELU1_BIAS_ANT"
    for _op in _dvo.OPS:
        if _op.name == name:
            return _op
    spec = _Spec(
        body=maxx(Src0 + C0, Zero) + minn(Src1, One),
        reference=lambda in0, in1, s0, s1, imm2:
            np.maximum(np.float32(in0) + s0, 0) + np.minimum(np.float32(in1), 1),
    )
    op = _dvo.DveOp(name, spec, subdim=False, uops_sha={})
    _dvo._SUB_OPCODE_FOR_NAME[name] = max(_dvo._SUB_OPCODE_FOR_NAME.values()) + 1
    import re
    for ver in ("v3", "v4"):
        try:
            op.compile(ver)
        except ValueError as e:
            m = re.search(ver + r': ([0-9a-f]+)', str(e))
            if m:
                op.uops_sha[ver] = m.group(1)
    _dvo.OPS.append(op)
    _dvo.CUSTOM_DVE_SPECS[name] = spec
    return op


ELU1 = _register_elu_op()

BF16 = mybir.dt.bfloat16
F32 = mybir.dt.float32
F8 = mybir.dt.float8e4
AF = mybir.ActivationFunctionType
ALU = mybir.AluOpType
DR = mybir.MatmulPerfMode.DoubleRow

N_CORES = 8
IMG = 2            # images per core (B=16)
C = 256
N = 4096           # pixels (64x64)
TAPS = [(ty - 1, tx - 1) for ty in range(3) for tx in range(3)]

# tap assignment: which tap indices run on PE (diag matmuls into at-psum,
# weights pre-scaled by 2^14) vs DVE (STT on [128,4096] lep tiles).
PE_TAPS = tuple(int(t) for t in os.environ.get("BK_PE_TAPS", "1,2,3,4,5,6,7,8").split(",") if t != "")
POOL_TAPS = tuple(int(t) for t in os.environ.get("BK_POOL_TAPS", "").split(",") if t != "")
DVE_TAPS = tuple(t for t in range(9) if t not in PE_TAPS and t not in POOL_TAPS)

_CACHE = {}


def build_program():
    nc = bacc.Bacc(
        "TRN2", target_bir_lowering=False, debug=False,
        enable_asserts=False, num_devices=N_CORES,
    )
    xb_d = nc.dram_tensor("xb", [IMG, 128, 8192], BF16, kind="ExternalInput").ap()
    xf_d = nc.dram_tensor("xf", [IMG, 128, 8192], F8, kind="ExternalInput").ap()
    w8_d = nc.dram_tensor("w8", [128, 1536], F8, kind="ExternalInput").ap()
    row8_d = nc.dram_tensor("row8", [1, 2560], F8, kind="ExternalInput").ap()
    wb_d = nc.dram_tensor("wb", [128, 3346], BF16, kind="ExternalInput").ap()
    em_d = nc.dram_tensor("em", [8, 256], BF16, kind="ExternalInput").ap()
    bcol_d = nc.dram_tensor("bcol", [128, 24], F32, kind="ExternalInput").ap()
    y_d = nc.dram_tensor("y", [IMG, C, N], BF16, kind="ExternalOutput").ap()

    with tile.TileContext(nc) as tc:
        with (
            tc.tile_pool(name="const", bufs=1) as const,
            tc.tile_pool(name="sb", bufs=1) as sb,
            tc.tile_pool(name="ps", bufs=1, space=bass.MemorySpace.PSUM) as ps,
        ):
            # ---------------- constants ----------------
            w8 = const.tile([128, 1536], F8, name="w8", tag="w8")
            nc.scalar.dma_start(w8[:], w8_d[:])
            wb = const.tile([128, 3346], BF16, name="wb", tag="wb")
            nc.scalar.dma_start(wb[:, 0:512], wb_d[:, 0:512])
            w83 = w8[:].rearrange("p (s f) -> p s f", s=2)
            wq8 = w83[:, :, 0:256]
            wk8 = w83[:, :, 256:512]
            wv8 = w83[:, :, 512:768]

            row8 = const.tile([1, 2560], F8, name="row8", tag="row8")
            nc.sync.dma_start(row8[:], row8_d[:])
            row83 = row8[:].rearrange("p (s f) -> p s f", s=2)
            ones8 = row83[:, :, 0:512]       # s0=1, s1=0
            bq8 = row83[:, :, 512:768]       # s0=bq, s1=0
            bk8 = row83[:, :, 768:1280]      # s0=[bk|bk], s1=0

            wvb = [wb[:, 0:256], wb[:, 256:512]]
            wo = [wb[:, 512:768], wb[:, 768:1024]]

            def diag(t, g):
                o = 1024 + (t * 2 + g) * 128
                return wb[:, o:o + 128]

            emt = const.tile([8, 256], BF16, name="emt", tag="emt")
            nc.sync.dma_start(emt[:], em_d[:])
            bct = const.tile([128, 24], F32, name="bct", tag="bct")
            nc.sync.dma_start(bct[:], bcol_d[:])
            bvc = [bct[:, 0:1], bct[:, 1:2]]
            btc = [bct[:, 2:3], bct[:, 3:4]]
            bqc = [bct[:, 22:23], bct[:, 23:24]]

            def wcol(t, g):
                o = 4 + t * 2 + g
                return bct[:, o:o + 1]

            ones4 = const.tile([128, 4], BF16, name="ones4", tag="ones4")
            nc.gpsimd.memset(ones4[:], 1.0)

            st = {}  # per-image state

            def load_x(u, img0):
                # split by pixel half, interleaved xf/xb so compute starts early
                xfd3 = xf_d[img0].rearrange("p (s f) -> p s f", s=2)
                xbd3 = xb_d[img0].rearrange("p (s f) -> p s f", s=2)
                xfh, xbh = [], []
                for hlf in range(2):
                    px = slice(2048 * hlf, 2048 * hlf + 2048)
                    tf = sb.tile([128, 4096], F8, name=f"xf{u}_{hlf}",
                                 tag=f"xf{hlf}", bufs=2)
                    nc.gpsimd.dma_start(
                        tf[:].rearrange("p (s f) -> p s f", s=2), xfd3[:, :, px])
                    xfh.append(tf[:].rearrange("p (s f) -> p s f", s=2))
                    tb = sb.tile([128, 4096], BF16, name=f"xb{u}_{hlf}",
                                 tag=f"xb{hlf}", bufs=2)
                    nc.gpsimd.dma_start(
                        tb[:].rearrange("p (s f) -> p s f", s=2), xbd3[:, :, px])
                    xbh.append(tb[:].rearrange("p (s f) -> p s f", s=2))
                if u == 0:
                    nc.scalar.dma_start(wb[:, 512:3346], wb_d[:, 512:3346])
                st[u] = {"xfh": xfh, "xbh": xbh}

                def xf3s(px0, n):
                    h = px0 // 2048
                    assert (px0 + n - 1) // 2048 == h
                    return xfh[h][:, :, px0 - 2048 * h:px0 - 2048 * h + n]

                def xb3s(gin, px0, n):
                    h = px0 // 2048
                    return xbh[h][:, gin, px0 - 2048 * h:px0 - 2048 * h + n]

                st[u]["xf3s"] = xf3s
                st[u]["xb3s"] = xb3s

            def phase_a(u):
                s_ = st[u]
                xf3s, xb3s = s_["xf3s"], s_["xb3s"]
                kvp = [ps.tile([128, 257], F32, name=f"kv{u}_{g}",
                               tag=f"kv{g}", bufs=1) for g in range(2)]
                s_["kvp"] = kvp
                vpad = []
                for g in range(2):
                    t = sb.tile([128, 66 * 66], BF16, name=f"vp{u}_{g}",
                                tag=f"vp{g}", bufs=2)
                    t3 = t[:].rearrange("p (r x) -> p r x", x=66)
                    # zero only the pad ring (interior is overwritten)
                    nc.gpsimd.memset(t3[:, 0:1, :], 0.0)
                    nc.gpsimd.memset(t3[:, 65:66, :], 0.0)
                    nc.gpsimd.memset(t3[:, 1:65, 0:1], 0.0)
                    nc.gpsimd.memset(t3[:, 1:65, 65:66], 0.0)
                    vpad.append(t)
                s_["vpad"] = vpad

                for j in range(16):
                    # ---- k pixel-major (fp8 DR), 2 chunks of 128 px ----
                    kp = ps.tile([128, 512], F32, name=f"kp{u}_{j}", tag="kp", bufs=3)
                    for h in range(2):
                        px0 = 128 * (2 * j + h)
                        nc.tensor.matmul(kp[:, 256 * h:256 * h + 256],
                                         xf3s(px0, 128), wk8,
                                         start=(h == 0), stop=False, perf_mode=DR)
                    nc.tensor.matmul(kp[:], ones8[:, :, 0:128], bk8,
                                     start=False, stop=True, perf_mode=DR)
                    ek = sb.tile([128, 512], BF16, name=f"ek{u}_{j}", tag="ek", bufs=8)
                    nc.scalar.activation(ek[:], kp[:], AF.Exp)
                    kh = sb.tile([128, 512], F8, name=f"kh{u}_{j}", tag="kh", bufs=8)
                    nc.vector._custom_dve(ELU1, out=kh[:], in0=kp[:], in1=ek[:])
                    kh3 = kh[:].rearrange("p (s f) -> p s f", s=2)

                    # ---- v pixel-major (fp8 DR) ----
                    vp = ps.tile([128, 512], F32, name=f"vq{u}_{j}", tag="vq", bufs=2)
                    for h in range(2):
                        px0 = 128 * (2 * j + h)
                        nc.tensor.matmul(vp[:, 256 * h:256 * h + 256],
                                         xf3s(px0, 128), wv8,
                                         start=(h == 0), stop=(h == 1), perf_mode=DR)
                    vsb = sb.tile([128, 514], F8, name=f"vs{u}_{j}", tag="vs", bufs=8)
                    vsb3 = vsb[:].rearrange("p (s f) -> p s f", s=2)
                    nc.gpsimd.memset(vsb3[:, :, 0:1], 1.0)
                    nc.scalar.activation(vsb3[:, :, 1:257], vp[:], AF.Identity,
                                         scale=0.25)

                    # ---- v channel-major (bf16) for LePE; one (g, c8) per j ----
                    g, c8 = j // 8, j % 8
                    vc = ps.tile([128, 512], F32, name=f"vc{u}_{j}", tag="vc", bufs=1)
                    for gin in range(2):
                        nc.tensor.matmul(vc[:], wvb[gin][:, 128 * g:128 * g + 128],
                                         xb3s(gin, 512 * c8, 512),
                                         start=(gin == 0), stop=(gin == 1))
                    vp3 = vpad[g][:].rearrange("p (r x) -> p r x", x=66)
                    dst = vp3[:, 8 * c8 + 1:8 * c8 + 9, 1:65]
                    nc.scalar.activation(dst, vc[:], AF.Identity, bias=bvc[g])

                    # ---- kv accumulation (fp8 DR over 256 px) ----
                    for g in range(2):
                        nc.tensor.matmul(kvp[g][:], kh3[:, :, 128 * g:128 * g + 128],
                                         vsb3, start=(j == 0), stop=(j == 15),
                                         perf_mode=DR)

                # ---- DVE lepe taps on [128, 4096] tiles ----
                if not DVE_TAPS and not POOL_TAPS:
                    s_["lep"] = None
                    return
                lep = []
                for g in range(2):
                    t = sb.tile([128, 4096], BF16, name=f"lp{u}_{g}",
                                tag=f"lp{g}", bufs=2)
                    t3 = t[:].rearrange("p (r x) -> p r x", x=64)
                    vp3 = vpad[g][:].rearrange("p (r x) -> p r x", x=66)
                    first = True
                    for ti in DVE_TAPS:
                        dy, dx = TAPS[ti]
                        win = vp3[:, dy + 1:dy + 65, dx + 1:dx + 65]
                        if first:
                            nc.vector.tensor_scalar_mul(t3[:], win, wcol(ti, g))
                            first = False
                        else:
                            nc.vector.scalar_tensor_tensor(t3[:], win, wcol(ti, g),
                                                           t3[:], ALU.mult, ALU.add)
                    for ti in POOL_TAPS:
                        dy, dx = TAPS[ti]
                        win = vp3[:, dy + 1:dy + 65, dx + 1:dx + 65]
                        nc.gpsimd.scalar_tensor_tensor(t3[:], win, wcol(ti, g),
                                                       t3[:], ALU.mult, ALU.add)
                    lep.append(t)
                s_["lep"] = lep

            def phase_b(u):
                s_ = st[u]
                kvp = s_["kvp"]
                bd = []
                for g in range(2):
                    t = sb.tile([128, 256], F8, name=f"bd{u}_{g}", tag=f"bd{g}", bufs=2)
                    nc.gpsimd.memset(t[:], 0.0)
                    t3 = t[:].rearrange("p (s f) -> p s f", s=2)
                    for h in range(4):
                        sl = slice(32 * h, 32 * h + 32)
                        c0 = 1 + 128 * g + 32 * h
                        nc.vector.tensor_copy(t3[sl, g, sl], kvp[g][sl, c0:c0 + 32])
                    bd.append(t)
                KS = sb.tile([128, 64], F8, name=f"KS{u}", tag="KS", bufs=2)
                nc.gpsimd.memset(KS[:], 0.0)
                KS4 = KS[:].rearrange("p (s r h) -> p s r h", s=2, h=8)
                for g in range(2):
                    for h4 in range(4):
                        sl = slice(32 * h4, 32 * h4 + 32)
                        h = 4 * g + h4
                        nc.vector.tensor_scalar(KS4[sl, g, :, h], ones4[sl, :],
                                                kvp[g][sl, 0:1], 1.0 / 512.0,
                                                ALU.mult, ALU.mult)
                s_["bd"], s_["KS"] = bd, KS

            def c_iter(u, img0, nt):
                s_ = st[u]
                xf3s = s_["xf3s"]
                bd, KS, lep, vpad = s_["bd"], s_["KS"], s_["lep"], s_["vpad"]
                KS3 = KS[:].rearrange("p (s f) -> p s f", s=2)  # [128, 2, 32]
                px = slice(512 * nt, 512 * nt + 512)

                S2 = sb.tile([128, 1024], F8, name=f"S{u}_{nt}", tag="S2", bufs=6)
                S23 = S2[:].rearrange("p (s f) -> p s f", s=2)
                for og in range(2):
                    qp = ps.tile([128, 512], F32, name=f"qp{u}_{og}_{nt}",
                                 tag="kp", bufs=3)
                    nc.tensor.matmul(qp[:], wq8[:, :, 128 * og:128 * og + 128],
                                     xf3s(512 * nt, 512), start=True, stop=True,
                                     perf_mode=DR)
                    eq = sb.tile([128, 512], BF16, name=f"eq{u}_{og}_{nt}",
                                 tag="eq", bufs=6)
                    nc.scalar.activation(eq[:], qp[:], AF.Exp, bias=bqc[og])
                    nc.vector._custom_dve(ELU1, out=S23[:, og, :], in0=qp[:],
                                          in1=eq[:], s0=bqc[og])

                den = ps.tile([32, 512], F32, name=f"dn{u}_{nt}", tag="kv0", bufs=1)
                nc.tensor.matmul(den[:], KS3, S23, start=True, stop=True, perf_mode=DR)
                rc = sb.tile([8, 512], BF16, name=f"rc{u}_{nt}", tag="rc", bufs=6)
                with nc.allow_low_precision(reason="recip feeds bf16 matmul"):
                    nc.vector.reciprocal(rc[:], den[0:8, :])

                qh = sb.tile([128, 1024], F8, name=f"qh{u}_{nt}", tag="qh", bufs=6)
                qh3 = qh[:].rearrange("p (s f) -> p s f", s=2)
                for g in range(2):
                    bc = ps.tile([128, 512], F32, name=f"bc{u}_{g}_{nt}",
                                 tag="kv0", bufs=1)
                    nc.tensor.matmul(bc[:], emt[:, 128 * g:128 * g + 128], rc[:],
                                     start=True, stop=True)
                    nc.vector.tensor_mul(qh3[:, g, :], S23[:, g, :], bc[:])
                rats = []
                for g in range(2):
                    bd3 = bd[g][:].rearrange("p (s f) -> p s f", s=2)
                    at = ps.tile([128, 512], F32, name=f"at{u}_{g}_{nt}",
                                 tag="vq", bufs=2)
                    vp3 = vpad[g][:].rearrange("p (r x) -> p r x", x=66)
                    for i, ti in enumerate(PE_TAPS):
                        dy, dx = TAPS[ti]
                        win = vp3[:, 8 * nt + dy + 1:8 * nt + dy + 9,
                                  dx + 1:dx + 65]
                        nc.tensor.matmul(at[:], diag(ti, g), win, start=(i == 0),
                                         stop=False)
                    nc.tensor.matmul(at[:], bd3, qh3,
                                     start=(len(PE_TAPS) == 0), stop=True,
                                     perf_mode=DR)
                    rat = sb.tile([128, 512], BF16, name=f"rt{u}_{g}_{nt}",
                                  tag=f"rt{g}", bufs=2)
                    if lep is not None:
                        nc.vector.scalar_tensor_tensor(rat[:], at[:],
                                                       1.0 / 16384.0,
                                                       lep[g][:][:, px],
                                                       ALU.mult, ALU.add)
                    else:
                        nc.vector.tensor_scalar_mul(rat[:], at[:], 1.0 / 16384.0)
                    rats.append(rat)

                for og in range(2):
                    op_ = ps.tile([128, 512], F32, name=f"op{u}_{og}_{nt}",
                                  tag="vc", bufs=1)
                    for g in range(2):
                        nc.tensor.matmul(op_[:], wo[g][:, 128 * og:128 * og + 128],
                                         rats[g][:], start=(g == 0), stop=(g == 1))
                    yt = sb.tile([128, 512], BF16, name=f"yt{u}_{og}_{nt}",
                                 tag="yt", bufs=6)
                    nc.scalar.activation(yt[:], op_[:], AF.Identity, bias=btc[og])
                    nc.sync.dma_start(
                        y_d[img0, 128 * og:128 * og + 128, px], yt[:])

            # ---------------- schedule ----------------
            us = list(range(IMG))
            for i, u in enumerate(us):
                load_x(u, i)
            for u in us:
                phase_a(u)
                phase_b(u)
            for nt in range(8):
                for i, u in enumerate(us):
                    c_iter(u, i, nt)

    nc.compile()
    return nc


def _prep_inputs(x, qkv_w, qkv_b, lepe_w, lepe_b, out_w, out_b):
    bf = ml_dtypes.bfloat16
    f8 = ml_dtypes.float8_e4m3
    x = np.ascontiguousarray(np.asarray(x, np.float32)).reshape(16, C, N)
    qkv_w = np.asarray(qkv_w, np.float32)
    qkv_b = np.asarray(qkv_b, np.float32)
    lepe_w = np.asarray(lepe_w, np.float32)
    lepe_b = np.asarray(lepe_b, np.float32)
    out_w = np.asarray(out_w, np.float32)
    out_b = np.asarray(out_b, np.float32)

    Wq, Wk, Wv = qkv_w[0:256], qkv_w[256:512], qkv_w[512:768]
    bq, bk, bv = qkv_b[0:256], qkv_b[256:512], qkv_b[512:768]

    # x in [p, s, n] layout: channel = s*128 + p
    xr = x.reshape(16, 2, 128, N).transpose(0, 2, 1, 3).reshape(16, 128, 2 * N)

    # w8: [128, 2, 768] = wq8 | wk8 | wv8, each [p, s, m] = W[m, s*128+p]
    w8 = np.zeros((128, 2, 768), np.float32)
    for s in range(2):
        w8[:, s, 0:256] = Wq[:, s * 128:(s + 1) * 128].T
        w8[:, s, 256:512] = Wk[:, s * 128:(s + 1) * 128].T
        w8[:, s, 512:768] = Wv[:, s * 128:(s + 1) * 128].T
    w8 = w8.reshape(128, 1536).astype(f8)

    # row8: [1, 2, 1280]: ones(512) | bq(256) | bkbk(512)
    row8 = np.zeros((1, 2, 1280), np.float32)
    row8[0, 0, 0:512] = 1.0
    row8[0, 0, 512:768] = bq
    row8[0, 0, 768:1280] = np.concatenate([bk, bk])
    row8 = row8.reshape(1, 2560).astype(f8)

    # wb (bf16): wvb(512) | wo(512) | diag taps(2304) | wcols(18)
    wb = np.zeros((128, 3346), np.float32)
    for gin in range(2):
        wb[:, 256 * gin:256 * gin + 256] = Wv[:, gin * 128:(gin + 1) * 128].T
        wb[:, 512 + 256 * gin:512 + 256 * gin + 256] = out_w[:, gin * 128:(gin + 1) * 128].T
    lw = lepe_w[:, 0]  # [256, 3, 3]
    for t, (dy, dx) in enumerate(TAPS):
        for g in range(2):
            o = 1024 + (t * 2 + g) * 128
            d = np.zeros((128, 128), np.float32)
            np.fill_diagonal(d, lw[g * 128:(g + 1) * 128, dy + 1, dx + 1] * 16384.0)
            wb[:, o:o + 128] = d
    wb = wb.astype(bf)

    em = np.zeros((8, 256), np.float32)
    for m in range(256):
        em[m // 32, m] = 128.0
    em = em.astype(bf)

    # v-bias columns + total output bias (v-bias in attn path folds into it)
    btot = out_b + out_w @ lepe_b + out_w @ bv
    bcol = np.zeros((128, 24), np.float32)
    bcol[:, 0] = bv[0:128]
    bcol[:, 1] = bv[128:256]
    bcol[:, 2] = btot[0:128]
    bcol[:, 3] = btot[128:256]
    for t in range(9):
        dy, dx = TAPS[t]
        for g in range(2):
            bcol[:, 4 + t * 2 + g] = lw[g * 128:(g + 1) * 128, dy + 1, dx + 1]
    bcol[:, 22] = bq[0:128]
    bcol[:, 23] = bq[128:256]

    shared = dict(w8=w8, row8=row8, wb=wb, em=em, bcol=bcol)
    in_maps = []
    for c in range(N_CORES):
        m = dict(shared)
        xc = xr[c * IMG:(c + 1) * IMG]
        m["xb"] = xc.astype(bf)
        m["xf"] = xc.astype(f8)
        in_maps.append(m)
    return in_maps


def kernel(x, qkv_w, qkv_b, lepe_w, lepe_b, out_w, out_b):
    if "nc" not in _CACHE:
        _CACHE["nc"] = build_program()
    nc = _CACHE["nc"]
    in_maps = _prep_inputs(x, qkv_w, qkv_b, lepe_w, lepe_b, out_w, out_b)
    res = run_bass_kernel_spmd(nc, in_maps, core_ids=list(range(N_CORES)))
    out = np.concatenate([np.asarray(r["y"], np.float32) for r in res.results])
    return out.reshape(16, C, 64, 64)


if __name__ == "__main__":
    build_program()
    print("build OK")


# revision 37
# speedup vs baseline: 1.0305x; 1.0132x over previous
"""Trainium2 Bass kernel for MultiHeadLinearSelfAttention (linear attention +
LePE depthwise conv + projections), SPMD data-parallel over batch on 8 cores.

v2 design (measured rel err 0.0058 vs 2e-2 gate; cost-model exec 165.5us/core
vs 215us baseline):

Precision split (LePE dominates output RMS 8.5:1 over attention):
  - LePE path bf16 end-to-end: v channel-major production, 3x3 taps,
    rat, out_proj; y output bf16.
  - Attention path fp8e4m3 with DoubleRow matmuls (2x PE rate): q/k
    production from an fp8 copy of x, kv accumulation (v pre-scaled 1/4),
    den (ksum/512, 4x-replicated stationary for the 32-partition ISA
    minimum), q-hat normalization, and the at = blockdiag(kv) @ qh matmul
    (qh scaled 2^16/den via em=128 and ksum/512; compensated by 2^-14 at
    the rat evacuation; LePE diag weights pre-scaled 2^14 to share the
    at-psum accumulation).

Engine balance (per core ~112us PE / ~103 DVE / ~99 ACT):
  - PE: all DR matmuls + bf16 vch/bc/out_proj + 8 of 9 LePE diag taps
    accumulated directly into at-psum (taps are emitted inside the at
    accumulation group, before the qh-dependent DR matmul).
  - ACT: Exp (k and q, q with fused per-partition bias), vsb 1/4-scale
    fp8 evacuation, vpad fill (+v bias), output bias evacuation.
  - DVE: custom fused op ELU1_BIAS_ANT: out = max(in0+s0,0)+min(in1,1)
    (one op for the elu tail, bias folded for q), qh mul, rat
    (at*2^-14 + lep STT), reciprocal, tap 0 as 4x tensor_scalar_mul.
  - Pool/GPSIMD: memsets + x DMAs only (hardware cannot touch PSUM from
    GPSIMD, and TensorScalarPtr STT/AP-scalar forms are PE/DVE-only).

Scheduling: per image, phase A (16 pair-iters: k/v pixel-major DR, elu,
kv accumulation, interleaved vch->vpad) then B (bd/KS extraction) emitted
before the next image's phase A (kv psum bufs=1 would otherwise stall PE);
phase C (8 chunks x 2 images interleaved). x is loaded as split
bf16/fp8 half-tiles so compute starts before the full image arrives.
"""

import os
import sys

for _p in ("/opt/trn_rl_repo",):
    if _p not in sys.path:
        sys.path.insert(0, _p)

import numpy as np
import ml_dtypes

import concourse.bass as bass
import concourse.bacc as bacc
import concourse.mybir as mybir
import concourse.tile as tile
from concourse.bass_utils import run_bass_kernel_spmd
from concourse import dve_ops as _dvo
from concourse.dve_spec import Spec as _Spec, Src0, Src1, C0, Zero, One, maxx, minn


def _register_elu_op():
    """Fused elu(x+b)+1 tail: out = max(in0+s0, 0) + min(in1, 1)."""
    name = "ELU1_BIAS_ANT"
    for _op in _dvo.OPS:
        if _op.name == name:
            return _op
    spec = _Spec(
        body=maxx(Src0 + C0, Zero) + minn(Src1, One),
        reference=lambda in0, in1, s0, s1, imm2:
            np.maximum(np.float32(in0) + s0, 0) + np.minimum(np.float32(in1), 1),
    )
    op = _dvo.DveOp(name, spec, subdim=False, uops_sha={})
    _dvo._SUB_OPCODE_FOR_NAME[name] = max(_dvo._SUB_OPCODE_FOR_NAME.values()) + 1
    import re
    for ver in ("v3", "v4"):
        try:
            op.compile(ver)
        except ValueError as e:
            m = re.search(ver + r': ([0-9a-f]+)', str(e))
            if m:
                op.uops_sha[ver] = m.group(1)
    _dvo.OPS.append(op)
    _dvo.CUSTOM_DVE_SPECS[name] = spec
    return op


ELU1 = _register_elu_op()

BF16 = mybir.dt.bfloat16
F32 = mybir.dt.float32
F8 = mybir.dt.float8e4
AF = mybir.ActivationFunctionType
ALU = mybir.AluOpType
DR = mybir.MatmulPerfMode.DoubleRow

N_CORES = 8
IMG = 2            # images per core (B=16)
C = 256
N = 4096           # pixels (64x64)
TAPS = [(ty - 1, tx - 1) for ty in range(3) for tx in range(3)]

# tap assignment: which tap indices run on PE (diag matmuls into at-psum,
# weights pre-scaled by 2^14) vs DVE (STT on [128,4096] lep tiles).
PE_TAPS = tuple(int(t) for t in os.environ.get("BK_PE_TAPS", "1,2,3,4,5,6,7,8").split(",") if t != "")
POOL_TAPS = tuple(int(t) for t in os.environ.get("BK_POOL_TAPS", "").split(",") if t != "")
DVE_TAPS = tuple(t for t in range(9) if t not in PE_TAPS and t not in POOL_TAPS)

_CACHE = {}


def build_program():
    nc = bacc.Bacc(
        "TRN2", target_bir_lowering=False, debug=False,
        enable_asserts=False, num_devices=N_CORES,
    )
    xb_d = nc.dram_tensor("xb", [IMG, 128, 8192], BF16, kind="ExternalInput").ap()
    xf_d = nc.dram_tensor("xf", [IMG, 128, 8192], F8, kind="ExternalInput").ap()
    w8_d = nc.dram_tensor("w8", [128, 1536], F8, kind="ExternalInput").ap()
    row8_d = nc.dram_tensor("row8", [1, 2560], F8, kind="ExternalInput").ap()
    wb_d = nc.dram_tensor("wb", [128, 3346], BF16, kind="ExternalInput").ap()
    em_d = nc.dram_tensor("em", [8, 256], BF16, kind="ExternalInput").ap()
    bcol_d = nc.dram_tensor("bcol", [128, 24], F32, kind="ExternalInput").ap()
    y_d = nc.dram_tensor("y", [IMG, C, N], BF16, kind="ExternalOutput").ap()

    with tile.TileContext(nc) as tc:
        with (
            tc.tile_pool(name="const", bufs=1) as const,
            tc.tile_pool(name="sb", bufs=1) as sb,
            tc.tile_pool(name="ps", bufs=1, space=bass.MemorySpace.PSUM) as ps,
        ):
            # ---------------- constants ----------------
            w8 = const.tile([128, 1536], F8, name="w8", tag="w8")
            nc.scalar.dma_start(w8[:], w8_d[:])
            wb = const.tile([128, 3346], BF16, name="wb", tag="wb")
            nc.scalar.dma_start(wb[:, 0:512], wb_d[:, 0:512])
            w83 = w8[:].rearrange("p (s f) -> p s f", s=2)
            wq8 = w83[:, :, 0:256]
            wk8 = w83[:, :, 256:512]
            wv8 = w83[:, :, 512:768]

            row8 = const.tile([1, 2560], F8, name="row8", tag="row8")
            nc.sync.dma_start(row8[:], row8_d[:])
            row83 = row8[:].rearrange("p (s f) -> p s f", s=2)
            ones8 = row83[:, :, 0:512]       # s0=1, s1=0
            bq8 = row83[:, :, 512:768]       # s0=bq, s1=0
            bk8 = row83[:, :, 768:1280]      # s0=[bk|bk], s1=0

            wvb = [wb[:, 0:256], wb[:, 256:512]]
            wo = [wb[:, 512:768], wb[:, 768:1024]]

            def diag(t, g):
                o = 1024 + (t * 2 + g) * 128
                return wb[:, o:o + 128]

            emt = const.tile([8, 256], BF16, name="emt", tag="emt")
            nc.sync.dma_start(emt[:], em_d[:])
            bct = const.tile([128, 24], F32, name="bct", tag="bct")
            nc.sync.dma_start(bct[:], bcol_d[:])
            bvc = [bct[:, 0:1], bct[:, 1:2]]
            btc = [bct[:, 2:3], bct[:, 3:4]]
            bqc = [bct[:, 22:23], bct[:, 23:24]]

            def wcol(t, g):
                o = 4 + t * 2 + g
                return bct[:, o:o + 1]

            ones4 = const.tile([128, 4], BF16, name="ones4", tag="ones4")
            nc.gpsimd.memset(ones4[:], 1.0)

            st = {}  # per-image state

            def load_x(u, img0):
                # split by pixel half, interleaved xf/xb so compute starts early
                xfd3 = xf_d[img0].rearrange("p (s f) -> p s f", s=2)
                xbd3 = xb_d[img0].rearrange("p (s f) -> p s f", s=2)
                xfh, xbh = [], []
                for hlf in range(2):
                    px = slice(2048 * hlf, 2048 * hlf + 2048)
                    tf = sb.tile([128, 4096], F8, name=f"xf{u}_{hlf}",
                                 tag=f"xf{hlf}", bufs=2)
                    nc.gpsimd.dma_start(
                        tf[:].rearrange("p (s f) -> p s f", s=2), xfd3[:, :, px])
                    xfh.append(tf[:].rearrange("p (s f) -> p s f", s=2))
                    tb = sb.tile([128, 4096], BF16, name=f"xb{u}_{hlf}",
                                 tag=f"xb{hlf}", bufs=2)
                    nc.gpsimd.dma_start(
                        tb[:].rearrange("p (s f) -> p s f", s=2), xbd3[:, :, px])
                    xbh.append(tb[:].rearrange("p (s f) -> p s f", s=2))
                if u == 0:
                    nc.scalar.dma_start(wb[:, 512:3346], wb_d[:, 512:3346])
                st[u] = {"xfh": xfh, "xbh": xbh}

                def xf3s(px0, n):
                    h = px0 // 2048
                    assert (px0 + n - 1) // 2048 == h
                    return xfh[h][:, :, px0 - 2048 * h:px0 - 2048 * h + n]

                def xb3s(gin, px0, n):
                    h = px0 // 2048
                    return xbh[h][:, gin, px0 - 2048 * h:px0 - 2048 * h + n]

                st[u]["xf3s"] = xf3s
                st[u]["xb3s"] = xb3s

            def phase_a(u):
                s_ = st[u]
                xf3s, xb3s = s_["xf3s"], s_["xb3s"]
                kvp = [ps.tile([128, 257], F32, name=f"kv{u}_{g}",
                               tag=f"kv{g}", bufs=1) for g in range(2)]
                s_["kvp"] = kvp
                vpad = []
                for g in range(2):
                    t = sb.tile([128, 66 * 66], BF16, name=f"vp{u}_{g}",
                                tag=f"vp{g}", bufs=2)
                    t3 = t[:].rearrange("p (r x) -> p r x", x=66)
                    # zero only the pad ring (interior is overwritten)
                    nc.gpsimd.memset(t3[:, 0:1, :], 0.0)
                    nc.gpsimd.memset(t3[:, 65:66, :], 0.0)
                    nc.gpsimd.memset(t3[:, 1:65, 0:1], 0.0)
                    nc.gpsimd.memset(t3[:, 1:65, 65:66], 0.0)
                    vpad.append(t)
                s_["vpad"] = vpad

                for j in range(16):
                    # ---- k pixel-major (fp8 DR), 2 chunks of 128 px ----
                    kp = ps.tile([128, 512], F32, name=f"kp{u}_{j}", tag="kp", bufs=3)
                    for h in range(2):
                        px0 = 128 * (2 * j + h)
                        nc.tensor.matmul(kp[:, 256 * h:256 * h + 256],
                                         xf3s(px0, 128), wk8,
                                         start=(h == 0), stop=False, perf_mode=DR)
                    nc.tensor.matmul(kp[:], ones8[:, :, 0:128], bk8,
                                     start=False, stop=True, perf_mode=DR)
                    ek = sb.tile([128, 512], BF16, name=f"ek{u}_{j}", tag="ek", bufs=10)
                    nc.scalar.activation(ek[:], kp[:], AF.Exp)
                    kh = sb.tile([128, 512], F8, name=f"kh{u}_{j}", tag="kh", bufs=10)
                    nc.vector._custom_dve(ELU1, out=kh[:], in0=kp[:], in1=ek[:])
                    kh3 = kh[:].rearrange("p (s f) -> p s f", s=2)

                    # ---- v pixel-major (fp8 DR) ----
                    vp = ps.tile([128, 512], F32, name=f"vq{u}_{j}", tag="vq", bufs=2)
                    for h in range(2):
                        px0 = 128 * (2 * j + h)
                        nc.tensor.matmul(vp[:, 256 * h:256 * h + 256],
                                         xf3s(px0, 128), wv8,
                                         start=(h == 0), stop=(h == 1), perf_mode=DR)
                    vsb = sb.tile([128, 514], F8, name=f"vs{u}_{j}", tag="vs", bufs=10)
                    vsb3 = vsb[:].rearrange("p (s f) -> p s f", s=2)
                    nc.gpsimd.memset(vsb3[:, :, 0:1], 1.0)
                    nc.scalar.activation(vsb3[:, :, 1:257], vp[:], AF.Identity,
                                         scale=0.25)

                    # ---- v channel-major (bf16) for LePE; one (g, c8) per j ----
                    g, c8 = j // 8, j % 8
                    vc = ps.tile([128, 512], F32, name=f"vc{u}_{j}", tag="vc", bufs=1)
                    for gin in range(2):
                        nc.tensor.matmul(vc[:], wvb[gin][:, 128 * g:128 * g + 128],
                                         xb3s(gin, 512 * c8, 512),
                                         start=(gin == 0), stop=(gin == 1))
                    vp3 = vpad[g][:].rearrange("p (r x) -> p r x", x=66)
                    dst = vp3[:, 8 * c8 + 1:8 * c8 + 9, 1:65]
                    nc.scalar.activation(dst, vc[:], AF.Identity, bias=bvc[g])

                    # ---- kv accumulation (fp8 DR over 256 px) ----
                    for g in range(2):
                        nc.tensor.matmul(kvp[g][:], kh3[:, :, 128 * g:128 * g + 128],
                                         vsb3, start=(j == 0), stop=(j == 15),
                                         perf_mode=DR)

                # ---- DVE lepe taps on [128, 4096] tiles ----
                if not DVE_TAPS and not POOL_TAPS:
                    s_["lep"] = None
                    return
                lep = []
                for g in range(2):
                    t = sb.tile([128, 4096], BF16, name=f"lp{u}_{g}",
                                tag=f"lp{g}", bufs=2)
                    t3 = t[:].rearrange("p (r x) -> p r x", x=64)
                    vp3 = vpad[g][:].rearrange("p (r x) -> p r x", x=66)
                    first = True
                    for ti in DVE_TAPS:
                        dy, dx = TAPS[ti]
                        win = vp3[:, dy + 1:dy + 65, dx + 1:dx + 65]
                        if first:
                            nc.vector.tensor_scalar_mul(t3[:], win, wcol(ti, g))
                            first = False
                        else:
                            nc.vector.scalar_tensor_tensor(t3[:], win, wcol(ti, g),
                                                           t3[:], ALU.mult, ALU.add)
                    for ti in POOL_TAPS:
                        dy, dx = TAPS[ti]
                        win = vp3[:, dy + 1:dy + 65, dx + 1:dx + 65]
                        nc.gpsimd.scalar_tensor_tensor(t3[:], win, wcol(ti, g),
                                                       t3[:], ALU.mult, ALU.add)
                    lep.append(t)
                s_["lep"] = lep

            def phase_b(u):
                s_ = st[u]
                kvp = s_["kvp"]
                bd = []
                for g in range(2):
                    t = sb.tile([128, 256], F8, name=f"bd{u}_{g}", tag=f"bd{g}", bufs=2)
                    nc.gpsimd.memset(t[:], 0.0)
                    t3 = t[:].rearrange("p (s f) -> p s f", s=2)
                    for h in range(4):
                        sl = slice(32 * h, 32 * h + 32)
                        c0 = 1 + 128 * g + 32 * h
                        nc.vector.tensor_copy(t3[sl, g, sl], kvp[g][sl, c0:c0 + 32])
                    bd.append(t)
                KS = sb.tile([128, 64], F8, name=f"KS{u}", tag="KS", bufs=2)
                nc.gpsimd.memset(KS[:], 0.0)
                KS4 = KS[:].rearrange("p (s r h) -> p s r h", s=2, h=8)
                for g in range(2):
                    for h4 in range(4):
                        sl = slice(32 * h4, 32 * h4 + 32)
                        h = 4 * g + h4
                        nc.vector.tensor_scalar(KS4[sl, g, :, h], ones4[sl, :],
                                                kvp[g][sl, 0:1], 1.0 / 512.0,
                                                ALU.mult, ALU.mult)
                s_["bd"], s_["KS"] = bd, KS

            def c_iter(u, img0, nt):
                s_ = st[u]
                xf3s = s_["xf3s"]
                bd, KS, lep, vpad = s_["bd"], s_["KS"], s_["lep"], s_["vpad"]
                KS3 = KS[:].rearrange("p (s f) -> p s f", s=2)  # [128, 2, 32]
                px = slice(512 * nt, 512 * nt + 512)

                S2 = sb.tile([128, 1024], F8, name=f"S{u}_{nt}", tag="S2", bufs=8)
                S23 = S2[:].rearrange("p (s f) -> p s f", s=2)
                for og in range(2):
                    qp = ps.tile([128, 512], F32, name=f"qp{u}_{og}_{nt}",
                                 tag="kp", bufs=3)
                    nc.tensor.matmul(qp[:], wq8[:, :, 128 * og:128 * og + 128],
                                     xf3s(512 * nt, 512), start=True, stop=True,
                                     perf_mode=DR)
                    eq = sb.tile([128, 512], BF16, name=f"eq{u}_{og}_{nt}",
                                 tag="eq", bufs=8)
                    nc.scalar.activation(eq[:], qp[:], AF.Exp, bias=bqc[og])
                    nc.vector._custom_dve(ELU1, out=S23[:, og, :], in0=qp[:],
                                          in1=eq[:], s0=bqc[og])

                den = ps.tile([32, 512], F32, name=f"dn{u}_{nt}", tag="kv0", bufs=1)
                nc.tensor.matmul(den[:], KS3, S23, start=True, stop=True, perf_mode=DR)
                rc = sb.tile([8, 512], BF16, name=f"rc{u}_{nt}", tag="rc", bufs=8)
                with nc.allow_low_precision(reason="recip feeds bf16 matmul"):
                    nc.vector.reciprocal(rc[:], den[0:8, :])

                qh = sb.tile([128, 1024], F8, name=f"qh{u}_{nt}", tag="qh", bufs=8)
                qh3 = qh[:].rearrange("p (s f) -> p s f", s=2)
                for g in range(2):
                    bc = ps.tile([128, 512], F32, name=f"bc{u}_{g}_{nt}",
                                 tag="kv0", bufs=1)
                    nc.tensor.matmul(bc[:], emt[:, 128 * g:128 * g + 128], rc[:],
                                     start=True, stop=True)
                    nc.vector.tensor_mul(qh3[:, g, :], S23[:, g, :], bc[:])
                rats = []
                for g in range(2):
                    bd3 = bd[g][:].rearrange("p (s f) -> p s f", s=2)
                    at = ps.tile([128, 512], F32, name=f"at{u}_{g}_{nt}",
                                 tag="vq", bufs=2)
                    vp3 = vpad[g][:].rearrange("p (r x) -> p r x", x=66)
                    for i, ti in enumerate(PE_TAPS):
                        dy, dx = TAPS[ti]
                        win = vp3[:, 8 * nt + dy + 1:8 * nt + dy + 9,
                                  dx + 1:dx + 65]
                        nc.tensor.matmul(at[:], diag(ti, g), win, start=(i == 0),
                                         stop=False)
                    nc.tensor.matmul(at[:], bd3, qh3,
                                     start=(len(PE_TAPS) == 0), stop=True,
                                     perf_mode=DR)
                    rat = sb.tile([128, 512], BF16, name=f"rt{u}_{g}_{nt}",
                                  tag=f"rt{g}", bufs=2)
                    if lep is not None:
                        nc.vector.scalar_tensor_tensor(rat[:], at[:],
                                                       1.0 / 16384.0,
                                                       lep[g][:][:, px],
                                                       ALU.mult, ALU.add)
                    else:
                        nc.vector.tensor_scalar_mul(rat[:], at[:], 1.0 / 16384.0)
                    rats.append(rat)

                for og in range(2):
                    op_ = ps.tile([128, 512], F32, name=f"op{u}_{og}_{nt}",
                                  tag="vc", bufs=1)
                    for g in range(2):
                        nc.tensor.matmul(op_[:], wo[g][:, 128 * og:128 * og + 128],
                                         rats[g][:], start=(g == 0), stop=(g == 1))
                    yt = sb.tile([128, 512], BF16, name=f"yt{u}_{og}_{nt}",
                                 tag="yt", bufs=8)
                    nc.scalar.activation(yt[:], op_[:], AF.Identity, bias=btc[og])
                    nc.sync.dma_start(
                        y_d[img0, 128 * og:128 * og + 128, px], yt[:])

            # ---------------- schedule ----------------
            us = list(range(IMG))
            for i, u in enumerate(us):
                load_x(u, i)
            for u in us:
                phase_a(u)
                phase_b(u)
            for nt in range(8):
                for i, u in enumerate(us):
                    c_iter(u, i, nt)

    nc.compile()
    return nc


def _prep_inputs(x, qkv_w, qkv_b, lepe_w, lepe_b, out_w, out_b):
    bf = ml_dtypes.bfloat16
    f8 = ml_dtypes.float8_e4m3
    x = np.ascontiguousarray(np.asarray(x, np.float32)).reshape(16, C, N)
    qkv_w = np.asarray(qkv_w, np.float32)
    qkv_b = np.asarray(qkv_b, np.float32)
    lepe_w = np.asarray(lepe_w, np.float32)
    lepe_b = np.asarray(lepe_b, np.float32)
    out_w = np.asarray(out_w, np.float32)
    out_b = np.asarray(out_b, np.float32)

    Wq, Wk, Wv = qkv_w[0:256], qkv_w[256:512], qkv_w[512:768]
    bq, bk, bv = qkv_b[0:256], qkv_b[256:512], qkv_b[512:768]

    # x in [p, s, n] layout: channel = s*128 + p
    xr = x.reshape(16, 2, 128, N).transpose(0, 2, 1, 3).reshape(16, 128, 2 * N)

    # w8: [128, 2, 768] = wq8 | wk8 | wv8, each [p, s, m] = W[m, s*128+p]
    w8 = np.zeros((128, 2, 768), np.float32)
    for s in range(2):
        w8[:, s, 0:256] = Wq[:, s * 128:(s + 1) * 128].T
        w8[:, s, 256:512] = Wk[:, s * 128:(s + 1) * 128].T
        w8[:, s, 512:768] = Wv[:, s * 128:(s + 1) * 128].T
    w8 = w8.reshape(128, 1536).astype(f8)

    # row8: [1, 2, 1280]: ones(512) | bq(256) | bkbk(512)
    row8 = np.zeros((1, 2, 1280), np.float32)
    row8[0, 0, 0:512] = 1.0
    row8[0, 0, 512:768] = bq
    row8[0, 0, 768:1280] = np.concatenate([bk, bk])
    row8 = row8.reshape(1, 2560).astype(f8)

    # wb (bf16): wvb(512) | wo(512) | diag taps(2304) | wcols(18)
    wb = np.zeros((128, 3346), np.float32)
    for gin in range(2):
        wb[:, 256 * gin:256 * gin + 256] = Wv[:, gin * 128:(gin + 1) * 128].T
        wb[:, 512 + 256 * gin:512 + 256 * gin + 256] = out_w[:, gin * 128:(gin + 1) * 128].T
    lw = lepe_w[:, 0]  # [256, 3, 3]
    for t, (dy, dx) in enumerate(TAPS):
        for g in range(2):
            o = 1024 + (t * 2 + g) * 128
            d = np.zeros((128, 128), np.float32)
            np.fill_diagonal(d, lw[g * 128:(g + 1) * 128, dy + 1, dx + 1] * 16384.0)
            wb[:, o:o + 128] = d
    wb = wb.astype(bf)

    em = np.zeros((8, 256), np.float32)
    for m in range(256):
        em[m // 32, m] = 128.0
    em = em.astype(bf)

    # v-bias columns + total output bias (v-bias in attn path folds into it)
    btot = out_b + out_w @ lepe_b + out_w @ bv
    bcol = np.zeros((128, 24), np.float32)
    bcol[:, 0] = bv[0:128]
    bcol[:, 1] = bv[128:256]
    bcol[:, 2] = btot[0:128]
    bcol[:, 3] = btot[128:256]
    for t in range(9):
        dy, dx = TAPS[t]
        for g in range(2):
            bcol[:, 4 + t * 2 + g] = lw[g * 128:(g + 1) * 128, dy + 1, dx + 1]
    bcol[:, 22] = bq[0:128]
    bcol[:, 23] = bq[128:256]

    shared = dict(w8=w8, row8=row8, wb=wb, em=em, bcol=bcol)
    in_maps = []
    for c in range(N_CORES):
        m = dict(shared)
        xc = xr[c * IMG:(c + 1) * IMG]
        m["xb"] = xc.astype(bf)
        m["xf"] = xc.astype(f8)
        in_maps.append(m)
    return in_maps


def kernel(x, qkv_w, qkv_b, lepe_w, lepe_b, out_w, out_b):
    if "nc" not in _CACHE:
        _CACHE["nc"] = build_program()
    nc = _CACHE["nc"]
    in_maps = _prep_inputs(x, qkv_w, qkv_b, lepe_w, lepe_b, out_w, out_b)
    res = run_bass_kernel_spmd(nc, in_maps, core_ids=list(range(N_CORES)))
    out = np.concatenate([np.asarray(r["y"], np.float32) for r in res.results])
    return out.reshape(16, C, 64, 64)


if __name__ == "__main__":
    build_program()
    print("build OK")


# revision 41
# speedup vs baseline: 1.0386x; 1.0079x over previous
"""Trainium2 Bass kernel for MultiHeadLinearSelfAttention (linear attention +
LePE depthwise conv + projections), SPMD data-parallel over batch on 8 cores.

v2 design (measured rel err 0.0058 vs 2e-2 gate; cost-model exec 163.4us/core
vs 215us baseline):

Precision split (LePE dominates output RMS 8.5:1 over attention):
  - LePE path bf16 end-to-end: v channel-major production, 3x3 taps,
    rat, out_proj; y output bf16.
  - Attention path fp8e4m3 with DoubleRow matmuls (2x PE rate): q/k
    production from an fp8 copy of x, kv accumulation (v pre-scaled 1/4),
    den (ksum/512, 4x-replicated stationary for the 32-partition ISA
    minimum), q-hat normalization, and the at = blockdiag(kv) @ qh matmul
    (qh scaled 2^16/den via em=128 and ksum/512; compensated by 2^-14 at
    the rat evacuation; LePE diag weights pre-scaled 2^14 to share the
    at-psum accumulation).

Engine balance (per core ~112us PE / ~103 DVE / ~99 ACT):
  - PE: all DR matmuls + bf16 vch/bc/out_proj + 8 of 9 LePE diag taps
    accumulated directly into at-psum (taps are emitted inside the at
    accumulation group, before the qh-dependent DR matmul).
  - ACT: Exp (k and q, q with fused per-partition bias), vsb 1/4-scale
    fp8 evacuation, vpad fill (+v bias), output bias evacuation.
  - DVE: custom fused op ELU1_BIAS_ANT: out = max(in0+s0,0)+min(in1,1)
    (one op for the elu tail, bias folded for q), qh mul, rat
    (at*2^-14 + lep STT), reciprocal, tap 0 as 4x tensor_scalar_mul.
  - Pool/GPSIMD: memsets + x DMAs only (hardware cannot touch PSUM from
    GPSIMD, and TensorScalarPtr STT/AP-scalar forms are PE/DVE-only).

Scheduling: per image, phase A (16 pair-iters: k/v pixel-major DR, elu,
kv accumulation, interleaved vch->vpad) then B (bd/KS extraction) emitted
before the next image's phase A (kv psum bufs=1 would otherwise stall PE);
phase C (8 chunks x 2 images interleaved). x is loaded as split
bf16/fp8 half-tiles so compute starts before the full image arrives.
"""

import os
import sys

for _p in ("/opt/trn_rl_repo",):
    if _p not in sys.path:
        sys.path.insert(0, _p)

import numpy as np
import ml_dtypes

import concourse.bass as bass
import concourse.bacc as bacc
import concourse.mybir as mybir
import concourse.tile as tile
from concourse.bass_utils import run_bass_kernel_spmd
from concourse import dve_ops as _dvo
from concourse.dve_spec import Spec as _Spec, Src0, Src1, C0, Zero, One, maxx, minn


def _register_elu_op():
    """Fused elu(x+b)+1 tail: out = max(in0+s0, 0) + min(in1, 1)."""
    name = "ELU1_BIAS_ANT"
    for _op in _dvo.OPS:
        if _op.name == name:
            return _op
    spec = _Spec(
        body=maxx(Src0 + C0, Zero) + minn(Src1, One),
        reference=lambda in0, in1, s0, s1, imm2:
            np.maximum(np.float32(in0) + s0, 0) + np.minimum(np.float32(in1), 1),
    )
    op = _dvo.DveOp(name, spec, subdim=False, uops_sha={})
    _dvo._SUB_OPCODE_FOR_NAME[name] = max(_dvo._SUB_OPCODE_FOR_NAME.values()) + 1
    import re
    for ver in ("v3", "v4"):
        try:
            op.compile(ver)
        except ValueError as e:
            m = re.search(ver + r': ([0-9a-f]+)', str(e))
            if m:
                op.uops_sha[ver] = m.group(1)
    _dvo.OPS.append(op)
    _dvo.CUSTOM_DVE_SPECS[name] = spec
    return op


ELU1 = _register_elu_op()

BF16 = mybir.dt.bfloat16
F32 = mybir.dt.float32
F8 = mybir.dt.float8e4
AF = mybir.ActivationFunctionType
ALU = mybir.AluOpType
DR = mybir.MatmulPerfMode.DoubleRow

N_CORES = 8
IMG = 2            # images per core (B=16)
C = 256
N = 4096           # pixels (64x64)
TAPS = [(ty - 1, tx - 1) for ty in range(3) for tx in range(3)]

# tap assignment: which tap indices run on PE (diag matmuls into at-psum,
# weights pre-scaled by 2^14) vs DVE (STT on [128,4096] lep tiles).
PE_TAPS = tuple(int(t) for t in os.environ.get("BK_PE_TAPS", "1,2,3,4,5,6,7,8").split(",") if t != "")
POOL_TAPS = tuple(int(t) for t in os.environ.get("BK_POOL_TAPS", "").split(",") if t != "")
DVE_TAPS = tuple(t for t in range(9) if t not in PE_TAPS and t not in POOL_TAPS)

_CACHE = {}


def build_program():
    nc = bacc.Bacc(
        "TRN2", target_bir_lowering=False, debug=False,
        enable_asserts=False, num_devices=N_CORES,
    )
    xb_d = nc.dram_tensor("xb", [IMG, 128, 8192], BF16, kind="ExternalInput").ap()
    xf_d = nc.dram_tensor("xf", [IMG, 128, 8192], F8, kind="ExternalInput").ap()
    w8_d = nc.dram_tensor("w8", [128, 1536], F8, kind="ExternalInput").ap()
    row8_d = nc.dram_tensor("row8", [1, 2560], F8, kind="ExternalInput").ap()
    wb_d = nc.dram_tensor("wb", [128, 3346], BF16, kind="ExternalInput").ap()
    em_d = nc.dram_tensor("em", [8, 256], BF16, kind="ExternalInput").ap()
    bcol_d = nc.dram_tensor("bcol", [128, 24], F32, kind="ExternalInput").ap()
    y_d = nc.dram_tensor("y", [IMG, C, N], BF16, kind="ExternalOutput").ap()

    with tile.TileContext(nc) as tc:
        with (
            tc.tile_pool(name="const", bufs=1) as const,
            tc.tile_pool(name="sb", bufs=1) as sb,
            tc.tile_pool(name="ps", bufs=1, space=bass.MemorySpace.PSUM) as ps,
        ):
            # ---------------- constants ----------------
            w8 = const.tile([128, 1536], F8, name="w8", tag="w8")
            nc.scalar.dma_start(w8[:], w8_d[:])
            wb = const.tile([128, 3346], BF16, name="wb", tag="wb")
            nc.scalar.dma_start(wb[:, 0:512], wb_d[:, 0:512])
            w83 = w8[:].rearrange("p (s f) -> p s f", s=2)
            wq8 = w83[:, :, 0:256]
            wk8 = w83[:, :, 256:512]
            wv8 = w83[:, :, 512:768]

            row8 = const.tile([1, 2560], F8, name="row8", tag="row8")
            nc.sync.dma_start(row8[:], row8_d[:])
            row83 = row8[:].rearrange("p (s f) -> p s f", s=2)
            ones8 = row83[:, :, 0:512]       # s0=1, s1=0
            bq8 = row83[:, :, 512:768]       # s0=bq, s1=0
            bk8 = row83[:, :, 768:1280]      # s0=[bk|bk], s1=0

            wvb = [wb[:, 0:256], wb[:, 256:512]]
            wo = [wb[:, 512:768], wb[:, 768:1024]]

            def diag(t, g):
                o = 1024 + (t * 2 + g) * 128
                return wb[:, o:o + 128]

            emt = const.tile([8, 256], BF16, name="emt", tag="emt")
            nc.sync.dma_start(emt[:], em_d[:])
            bct = const.tile([128, 24], F32, name="bct", tag="bct")
            nc.sync.dma_start(bct[:], bcol_d[:])
            bvc = [bct[:, 0:1], bct[:, 1:2]]
            btc = [bct[:, 2:3], bct[:, 3:4]]
            bqc = [bct[:, 22:23], bct[:, 23:24]]

            def wcol(t, g):
                o = 4 + t * 2 + g
                return bct[:, o:o + 1]

            ones4 = const.tile([128, 4], BF16, name="ones4", tag="ones4")
            nc.gpsimd.memset(ones4[:], 1.0)

            st = {}  # per-image state

            def load_x(u, img0):
                # split by pixel half, interleaved xf/xb so compute starts early
                xfd3 = xf_d[img0].rearrange("p (s f) -> p s f", s=2)
                xbd3 = xb_d[img0].rearrange("p (s f) -> p s f", s=2)
                xfh, xbh = [], []
                for hlf in range(2):
                    px = slice(2048 * hlf, 2048 * hlf + 2048)
                    tf = sb.tile([128, 4096], F8, name=f"xf{u}_{hlf}",
                                 tag=f"xf{hlf}", bufs=2)
                    nc.gpsimd.dma_start(
                        tf[:].rearrange("p (s f) -> p s f", s=2), xfd3[:, :, px])
                    xfh.append(tf[:].rearrange("p (s f) -> p s f", s=2))
                    tb = sb.tile([128, 4096], BF16, name=f"xb{u}_{hlf}",
                                 tag=f"xb{hlf}", bufs=2)
                    nc.gpsimd.dma_start(
                        tb[:].rearrange("p (s f) -> p s f", s=2), xbd3[:, :, px])
                    xbh.append(tb[:].rearrange("p (s f) -> p s f", s=2))
                if u == 0:
                    nc.scalar.dma_start(wb[:, 512:3346], wb_d[:, 512:3346])
                st[u] = {"xfh": xfh, "xbh": xbh}

                def xf3s(px0, n):
                    h = px0 // 2048
                    assert (px0 + n - 1) // 2048 == h
                    return xfh[h][:, :, px0 - 2048 * h:px0 - 2048 * h + n]

                def xb3s(gin, px0, n):
                    h = px0 // 2048
                    return xbh[h][:, gin, px0 - 2048 * h:px0 - 2048 * h + n]

                st[u]["xf3s"] = xf3s
                st[u]["xb3s"] = xb3s

            def phase_a(u):
                s_ = st[u]
                xf3s, xb3s = s_["xf3s"], s_["xb3s"]
                kvp = [ps.tile([128, 257], F32, name=f"kv{u}_{g}",
                               tag=f"kv{g}", bufs=1) for g in range(2)]
                s_["kvp"] = kvp
                vpad = []
                for g in range(2):
                    t = sb.tile([128, 66 * 66], BF16, name=f"vp{u}_{g}",
                                tag=f"vp{g}", bufs=2)
                    t3 = t[:].rearrange("p (r x) -> p r x", x=66)
                    # zero only the pad ring (interior is overwritten)
                    nc.gpsimd.memset(t3[:, 0:1, :], 0.0)
                    nc.gpsimd.memset(t3[:, 65:66, :], 0.0)
                    nc.gpsimd.memset(t3[:, 1:65, 0:1], 0.0)
                    nc.gpsimd.memset(t3[:, 1:65, 65:66], 0.0)
                    vpad.append(t)
                s_["vpad"] = vpad

                for j in range(16):
                    # ---- k pixel-major (fp8 DR), 2 chunks of 128 px ----
                    kp = ps.tile([128, 512], F32, name=f"kp{u}_{j}", tag="kp", bufs=3)
                    for h in range(2):
                        px0 = 128 * (2 * j + h)
                        nc.tensor.matmul(kp[:, 256 * h:256 * h + 256],
                                         xf3s(px0, 128), wk8,
                                         start=(h == 0), stop=False, perf_mode=DR)
                    nc.tensor.matmul(kp[:], ones8[:, :, 0:128], bk8,
                                     start=False, stop=True, perf_mode=DR)
                    ek = sb.tile([128, 512], BF16, name=f"ek{u}_{j}", tag="ek", bufs=12)
                    nc.scalar.activation(ek[:], kp[:], AF.Exp)
                    kh = sb.tile([128, 512], F8, name=f"kh{u}_{j}", tag="kh", bufs=12)
                    nc.vector._custom_dve(ELU1, out=kh[:], in0=kp[:], in1=ek[:])
                    kh3 = kh[:].rearrange("p (s f) -> p s f", s=2)

                    # ---- v pixel-major (fp8 DR) ----
                    vp = ps.tile([128, 512], F32, name=f"vq{u}_{j}", tag="vq", bufs=2)
                    for h in range(2):
                        px0 = 128 * (2 * j + h)
                        nc.tensor.matmul(vp[:, 256 * h:256 * h + 256],
                                         xf3s(px0, 128), wv8,
                                         start=(h == 0), stop=(h == 1), perf_mode=DR)
                    vsb = sb.tile([128, 514], F8, name=f"vs{u}_{j}", tag="vs", bufs=12)
                    vsb3 = vsb[:].rearrange("p (s f) -> p s f", s=2)
                    nc.gpsimd.memset(vsb3[:, :, 0:1], 1.0)
                    nc.scalar.activation(vsb3[:, :, 1:257], vp[:], AF.Identity,
                                         scale=0.25)

                    # ---- v channel-major (bf16) for LePE; one (g, c8) per j ----
                    g, c8 = j // 8, j % 8
                    vc = ps.tile([128, 512], F32, name=f"vc{u}_{j}", tag="vc", bufs=1)
                    for gin in range(2):
                        nc.tensor.matmul(vc[:], wvb[gin][:, 128 * g:128 * g + 128],
                                         xb3s(gin, 512 * c8, 512),
                                         start=(gin == 0), stop=(gin == 1))
                    vp3 = vpad[g][:].rearrange("p (r x) -> p r x", x=66)
                    dst = vp3[:, 8 * c8 + 1:8 * c8 + 9, 1:65]
                    nc.scalar.activation(dst, vc[:], AF.Identity, bias=bvc[g])

                    # ---- kv accumulation (fp8 DR over 256 px) ----
                    for g in range(2):
                        nc.tensor.matmul(kvp[g][:], kh3[:, :, 128 * g:128 * g + 128],
                                         vsb3, start=(j == 0), stop=(j == 15),
                                         perf_mode=DR)

                # ---- DVE lepe taps on [128, 4096] tiles ----
                if not DVE_TAPS and not POOL_TAPS:
                    s_["lep"] = None
                    return
                lep = []
                for g in range(2):
                    t = sb.tile([128, 4096], BF16, name=f"lp{u}_{g}",
                                tag=f"lp{g}", bufs=2)
                    t3 = t[:].rearrange("p (r x) -> p r x", x=64)
                    vp3 = vpad[g][:].rearrange("p (r x) -> p r x", x=66)
                    first = True
                    for ti in DVE_TAPS:
                        dy, dx = TAPS[ti]
                        win = vp3[:, dy + 1:dy + 65, dx + 1:dx + 65]
                        if first:
                            nc.vector.tensor_scalar_mul(t3[:], win, wcol(ti, g))
                            first = False
                        else:
                            nc.vector.scalar_tensor_tensor(t3[:], win, wcol(ti, g),
                                                           t3[:], ALU.mult, ALU.add)
                    for ti in POOL_TAPS:
                        dy, dx = TAPS[ti]
                        win = vp3[:, dy + 1:dy + 65, dx + 1:dx + 65]
                        nc.gpsimd.scalar_tensor_tensor(t3[:], win, wcol(ti, g),
                                                       t3[:], ALU.mult, ALU.add)
                    lep.append(t)
                s_["lep"] = lep

            def phase_b(u):
                s_ = st[u]
                kvp = s_["kvp"]
                bd = []
                for g in range(2):
                    t = sb.tile([128, 256], F8, name=f"bd{u}_{g}", tag=f"bd{g}", bufs=2)
                    nc.gpsimd.memset(t[:], 0.0)
                    t3 = t[:].rearrange("p (s f) -> p s f", s=2)
                    for h in range(4):
                        sl = slice(32 * h, 32 * h + 32)
                        c0 = 1 + 128 * g + 32 * h
                        nc.vector.tensor_copy(t3[sl, g, sl], kvp[g][sl, c0:c0 + 32])
                    bd.append(t)
                KS = sb.tile([128, 64], F8, name=f"KS{u}", tag="KS", bufs=2)
                nc.gpsimd.memset(KS[:], 0.0)
                KS4 = KS[:].rearrange("p (s r h) -> p s r h", s=2, h=8)
                for g in range(2):
                    for h4 in range(4):
                        sl = slice(32 * h4, 32 * h4 + 32)
                        h = 4 * g + h4
                        nc.vector.tensor_scalar(KS4[sl, g, :, h], ones4[sl, :],
                                                kvp[g][sl, 0:1], 1.0 / 512.0,
                                                ALU.mult, ALU.mult)
                s_["bd"], s_["KS"] = bd, KS

            def c_iter(u, img0, nt):
                s_ = st[u]
                xf3s = s_["xf3s"]
                bd, KS, lep, vpad = s_["bd"], s_["KS"], s_["lep"], s_["vpad"]
                KS3 = KS[:].rearrange("p (s f) -> p s f", s=2)  # [128, 2, 32]
                px = slice(512 * nt, 512 * nt + 512)

                S2 = sb.tile([128, 1024], F8, name=f"S{u}_{nt}", tag="S2", bufs=10)
                S23 = S2[:].rearrange("p (s f) -> p s f", s=2)
                for og in range(2):
                    qp = ps.tile([128, 512], F32, name=f"qp{u}_{og}_{nt}",
                                 tag="kp", bufs=3)
                    nc.tensor.matmul(qp[:], wq8[:, :, 128 * og:128 * og + 128],
                                     xf3s(512 * nt, 512), start=True, stop=True,
                                     perf_mode=DR)
                    eq = sb.tile([128, 512], BF16, name=f"eq{u}_{og}_{nt}",
                                 tag="eq", bufs=10)
                    nc.scalar.activation(eq[:], qp[:], AF.Exp, bias=bqc[og])
                    nc.vector._custom_dve(ELU1, out=S23[:, og, :], in0=qp[:],
                                          in1=eq[:], s0=bqc[og])

                den = ps.tile([32, 512], F32, name=f"dn{u}_{nt}", tag="kv0", bufs=1)
                nc.tensor.matmul(den[:], KS3, S23, start=True, stop=True, perf_mode=DR)
                rc = sb.tile([8, 512], BF16, name=f"rc{u}_{nt}", tag="rc", bufs=10)
                with nc.allow_low_precision(reason="recip feeds bf16 matmul"):
                    nc.vector.reciprocal(rc[:], den[0:8, :])

                qh = sb.tile([128, 1024], F8, name=f"qh{u}_{nt}", tag="qh", bufs=10)
                qh3 = qh[:].rearrange("p (s f) -> p s f", s=2)
                for g in range(2):
                    bc = ps.tile([128, 512], F32, name=f"bc{u}_{g}_{nt}",
                                 tag="kv0", bufs=1)
                    nc.tensor.matmul(bc[:], emt[:, 128 * g:128 * g + 128], rc[:],
                                     start=True, stop=True)
                    nc.vector.tensor_mul(qh3[:, g, :], S23[:, g, :], bc[:])
                rats = []
                for g in range(2):
                    bd3 = bd[g][:].rearrange("p (s f) -> p s f", s=2)
                    at = ps.tile([128, 512], F32, name=f"at{u}_{g}_{nt}",
                                 tag="vq", bufs=2)
                    vp3 = vpad[g][:].rearrange("p (r x) -> p r x", x=66)
                    for i, ti in enumerate(PE_TAPS):
                        dy, dx = TAPS[ti]
                        win = vp3[:, 8 * nt + dy + 1:8 * nt + dy + 9,
                                  dx + 1:dx + 65]
                        nc.tensor.matmul(at[:], diag(ti, g), win, start=(i == 0),
                                         stop=False)
                    nc.tensor.matmul(at[:], bd3, qh3,
                                     start=(len(PE_TAPS) == 0), stop=True,
                                     perf_mode=DR)
                    rat = sb.tile([128, 512], BF16, name=f"rt{u}_{g}_{nt}",
                                  tag=f"rt{g}", bufs=2)
                    if lep is not None:
                        nc.vector.scalar_tensor_tensor(rat[:], at[:],
                                                       1.0 / 16384.0,
                                                       lep[g][:][:, px],
                                                       ALU.mult, ALU.add)
                    else:
                        nc.vector.tensor_scalar_mul(rat[:], at[:], 1.0 / 16384.0)
                    rats.append(rat)

                for og in range(2):
                    op_ = ps.tile([128, 512], F32, name=f"op{u}_{og}_{nt}",
                                  tag="vc", bufs=1)
                    for g in range(2):
                        nc.tensor.matmul(op_[:], wo[g][:, 128 * og:128 * og + 128],
                                         rats[g][:], start=(g == 0), stop=(g == 1))
                    yt = sb.tile([128, 512], BF16, name=f"yt{u}_{og}_{nt}",
                                 tag="yt", bufs=10)
                    nc.scalar.activation(yt[:], op_[:], AF.Identity, bias=btc[og])
                    nc.sync.dma_start(
                        y_d[img0, 128 * og:128 * og + 128, px], yt[:])

            # ---------------- schedule ----------------
            us = list(range(IMG))
            for i, u in enumerate(us):
                load_x(u, i)
            for u in us:
                phase_a(u)
                phase_b(u)
            for nt in range(8):
                for i, u in enumerate(us):
                    c_iter(u, i, nt)

    nc.compile()
    return nc


def _prep_inputs(x, qkv_w, qkv_b, lepe_w, lepe_b, out_w, out_b):
    bf = ml_dtypes.bfloat16
    f8 = ml_dtypes.float8_e4m3
    x = np.ascontiguousarray(np.asarray(x, np.float32)).reshape(16, C, N)
    qkv_w = np.asarray(qkv_w, np.float32)
    qkv_b = np.asarray(qkv_b, np.float32)
    lepe_w = np.asarray(lepe_w, np.float32)
    lepe_b = np.asarray(lepe_b, np.float32)
    out_w = np.asarray(out_w, np.float32)
    out_b = np.asarray(out_b, np.float32)

    Wq, Wk, Wv = qkv_w[0:256], qkv_w[256:512], qkv_w[512:768]
    bq, bk, bv = qkv_b[0:256], qkv_b[256:512], qkv_b[512:768]

    # x in [p, s, n] layout: channel = s*128 + p
    xr = x.reshape(16, 2, 128, N).transpose(0, 2, 1, 3).reshape(16, 128, 2 * N)

    # w8: [128, 2, 768] = wq8 | wk8 | wv8, each [p, s, m] = W[m, s*128+p]
    w8 = np.zeros((128, 2, 768), np.float32)
    for s in range(2):
        w8[:, s, 0:256] = Wq[:, s * 128:(s + 1) * 128].T
        w8[:, s, 256:512] = Wk[:, s * 128:(s + 1) * 128].T
        w8[:, s, 512:768] = Wv[:, s * 128:(s + 1) * 128].T
    w8 = w8.reshape(128, 1536).astype(f8)

    # row8: [1, 2, 1280]: ones(512) | bq(256) | bkbk(512)
    row8 = np.zeros((1, 2, 1280), np.float32)
    row8[0, 0, 0:512] = 1.0
    row8[0, 0, 512:768] = bq
    row8[0, 0, 768:1280] = np.concatenate([bk, bk])
    row8 = row8.reshape(1, 2560).astype(f8)

    # wb (bf16): wvb(512) | wo(512) | diag taps(2304) | wcols(18)
    wb = np.zeros((128, 3346), np.float32)
    for gin in range(2):
        wb[:, 256 * gin:256 * gin + 256] = Wv[:, gin * 128:(gin + 1) * 128].T
        wb[:, 512 + 256 * gin:512 + 256 * gin + 256] = out_w[:, gin * 128:(gin + 1) * 128].T
    lw = lepe_w[:, 0]  # [256, 3, 3]
    for t, (dy, dx) in enumerate(TAPS):
        for g in range(2):
            o = 1024 + (t * 2 + g) * 128
            d = np.zeros((128, 128), np.float32)
            np.fill_diagonal(d, lw[g * 128:(g + 1) * 128, dy + 1, dx + 1] * 16384.0)
            wb[:, o:o + 128] = d
    wb = wb.astype(bf)

    em = np.zeros((8, 256), np.float32)
    for m in range(256):
        em[m // 32, m] = 128.0
    em = em.astype(bf)

    # v-bias columns + total output bias (v-bias in attn path folds into it)
    btot = out_b + out_w @ lepe_b + out_w @ bv
    bcol = np.zeros((128, 24), np.float32)
    bcol[:, 0] = bv[0:128]
    bcol[:, 1] = bv[128:256]
    bcol[:, 2] = btot[0:128]
    bcol[:, 3] = btot[128:256]
    for t in range(9):
        dy, dx = TAPS[t]
        for g in range(2):
            bcol[:, 4 + t * 2 + g] = lw[g * 128:(g + 1) * 128, dy + 1, dx + 1]
    bcol[:, 22] = bq[0:128]
    bcol[:, 23] = bq[128:256]

    shared = dict(w8=w8, row8=row8, wb=wb, em=em, bcol=bcol)
    in_maps = []
    for c in range(N_CORES):
        m = dict(shared)
        xc = xr[c * IMG:(c + 1) * IMG]
        m["xb"] = xc.astype(bf)
        m["xf"] = xc.astype(f8)
        in_maps.append(m)
    return in_maps


def kernel(x, qkv_w, qkv_b, lepe_w, lepe_b, out_w, out_b):
    if "nc" not in _CACHE:
        _CACHE["nc"] = build_program()
    nc = _CACHE["nc"]
    in_maps = _prep_inputs(x, qkv_w, qkv_b, lepe_w, lepe_b, out_w, out_b)
    res = run_bass_kernel_spmd(nc, in_maps, core_ids=list(range(N_CORES)))
    out = np.concatenate([np.asarray(r["y"], np.float32) for r in res.results])
    return out.reshape(16, C, 64, 64)


if __name__ == "__main__":
    build_program()
    print("build OK")
